# revision 1
# baseline (speedup 1.0000x reference)
"""Trainium2 Bass kernel for a dense transformer block (B=2,S=2048,D=1024,H=16,DFF=4096).

Sharding across 8 NeuronCores:
  core c: batch b=c//4, group rank r=c%4, replica groups [[0,1,2,3],[4,5,6,7]].
  - LN1 computed on own 512-token strip; hT AllGather'ed within the 4-core group.
  - Attention: head-parallel (4 heads/core, full causal sequence).
  - AllToAll redistributes attention output from head-sharded to token-sharded.
  - out_proj, LN2, FFN: token-sharded (512 tokens/core), full weights.
Matmuls run in float32r (TF32-like, full PE rate); the residual spine stays fp32.
"""
import sys

sys.path.insert(0, "/opt/trn_rl_repo")

import numpy as np

import concourse.bass as bass
import concourse.mybir as mybir
import concourse.tile as tile
from concourse import bacc
from concourse.bass_utils import run_bass_kernel_spmd
from concourse.masks import make_identity

AF = mybir.ActivationFunctionType
ALU = mybir.AluOpType
F32 = mybir.dt.float32
F32R = mybir.dt.float32r

B, S, D, H = 2, 2048, 1024, 16
DH = D // H          # 64
DFF = 4 * D          # 4096
EPS = 1e-5
NC = 8               # cores
G = 4                # cores per group (per batch)
TS = S // G          # 512 tokens per core
HC = H // G          # 4 heads per core
CC = HC * DH         # 256 head-columns per core
P = 128
KD = D // P          # 8 k-tiles over D
KF = DFF // P        # 32 k-tiles over DFF
NT = TS // P         # 4 token tiles per strip
GROUPS = [[0, 1, 2, 3], [4, 5, 6, 7]]

_CACHE = {}


def build():
    nc = bacc.Bacc(None)

    io = {}
    io["x_d"] = nc.declare_dram_parameter("x", [TS, D], F32, isOutput=False)
    io["ln1g_d"] = nc.declare_dram_parameter("ln1_g", [D], F32, isOutput=False)
    io["ln1b_d"] = nc.declare_dram_parameter("ln1_b", [D], F32, isOutput=False)
    io["wq_d"] = nc.declare_dram_parameter("Wq", [D, CC], F32R, isOutput=False)
    io["wk_d"] = nc.declare_dram_parameter("Wk", [D, CC], F32R, isOutput=False)
    io["wv_d"] = nc.declare_dram_parameter("Wv", [D, CC], F32R, isOutput=False)
    io["bq_d"] = nc.declare_dram_parameter("bq", [CC], F32R, isOutput=False)
    io["bk_d"] = nc.declare_dram_parameter("bk", [CC], F32R, isOutput=False)
    io["bv_d"] = nc.declare_dram_parameter("bv", [CC], F32R, isOutput=False)
    io["wo_d"] = nc.declare_dram_parameter("Wo", [D, D], F32R, isOutput=False)
    io["bo_d"] = nc.declare_dram_parameter("bo", [D], F32R, isOutput=False)
    io["ln2g_d"] = nc.declare_dram_parameter("ln2_g", [D], F32, isOutput=False)
    io["ln2b_d"] = nc.declare_dram_parameter("ln2_b", [D], F32, isOutput=False)
    io["w1_d"] = nc.declare_dram_parameter("W1", [D, DFF], F32R, isOutput=False)
    io["b1_d"] = nc.declare_dram_parameter("b1", [DFF], F32R, isOutput=False)
    io["w2_d"] = nc.declare_dram_parameter("W2", [DFF, D], F32R, isOutput=False)
    io["b2_d"] = nc.declare_dram_parameter("b2", [D], F32R, isOutput=False)
    io["y_d"] = nc.declare_dram_parameter("y", [TS, D], F32, isOutput=True)

    io["ag1_in"] = nc.dram_tensor("ag1_in", [D, TS], F32R)
    io["ag1_out"] = nc.dram_tensor("ag1_out", [G, D, TS], F32R)
    io["a2a_in"] = nc.dram_tensor("a2a_in", [CC, S], F32R)
    io["a2a_out"] = nc.dram_tensor("a2a_out", [G, CC, S], F32R)
    io["coff_d"] = nc.declare_dram_parameter("coff", [1, 1], mybir.dt.int32,
                                             isOutput=False)

    with tile.TileContext(nc) as tc:
        _body(nc, tc, io)
    nc.compile()
    return nc


def _body(nc, tc, t):
    with tc.tile_pool(name="const", bufs=1) as cst:
        # x strip loads first: they gate the LN1 -> transpose -> AG1 chain
        xsp_cm = tc.tile_pool(name="xsP", bufs=1)
        xsp = xsp_cm.__enter__()
        xs = [xsp.tile([P, D], F32, tag=f"xs{mt}", name=f"xs{mt}")
              for mt in range(NT)]
        for mt in range(NT):
            nc.sync.dma_start(xs[mt][:], t["x_d"][mt * P:(mt + 1) * P, :])

        # ---------------- constants ----------------
        ident = cst.tile([P, P], F32)
        make_identity(nc, ident[:])

        onesrow_f = cst.tile([1, TS], F32)
        nc.gpsimd.memset(onesrow_f[:], 1.0)
        ones128 = cst.tile([1, P], F32R)        # K=1 lhsT (M=128 tokens)
        nc.vector.tensor_copy(ones128[:], onesrow_f[0:1, 0:P])
        onescol4 = cst.tile([P, HC, 1], F32)
        nc.gpsimd.memset(onescol4[:], 1.0)
        epsc = cst.tile([P, 1], F32)
        nc.gpsimd.memset(epsc[:], EPS)

        # doubled causal masks (one per diagonal shift), mask||mask layout so a
        # single DVE op masks a two-head [128, 1024] pair tile.
        maskd = {}
        for sh in (0, -128, -256, -384):
            md = cst.tile([P, 2 * TS], F32, tag=f"maskd{sh}", name=f"maskd{sh}")
            nc.gpsimd.memset(md[:], 1.0)
            for half in range(2):
                nc.gpsimd.affine_select(
                    out=md[:, half * TS:(half + 1) * TS],
                    in_=md[:, half * TS:(half + 1) * TS],
                    compare_op=ALU.is_ge, fill=0.0, base=sh,
                    pattern=[[1, TS]], channel_multiplier=-1,
                )
            maskd[sh] = md

        # layernorm gains/biases as [128, KD] (per-partition per k-tile)
        ln1g = cst.tile([P, KD], F32)
        ln1b = cst.tile([P, KD], F32)
        ln2g = cst.tile([P, KD], F32)
        ln2b = cst.tile([P, KD], F32)
        nc.sync.dma_start(ln1g[:], t["ln1g_d"].rearrange("(k p) -> p k", p=P))
        nc.sync.dma_start(ln1b[:], t["ln1b_d"].rearrange("(k p) -> p k", p=P))
        nc.sync.dma_start(ln2g[:], t["ln2g_d"].rearrange("(k p) -> p k", p=P))
        nc.sync.dma_start(ln2b[:], t["ln2b_d"].rearrange("(k p) -> p k", p=P))

        # bq/bk as per-partition [128, 2] (column-tile-major) for psum eviction
        bqp = cst.tile([P, 2], F32)
        bkp = cst.tile([P, 2], F32)
        nc.gpsimd.dma_start(bqp[:], t["bq_d"].rearrange("(m p) -> p m", p=P))
        nc.gpsimd.dma_start(bkp[:], t["bk_d"].rearrange("(m p) -> p m", p=P))
        # bv broadcast across partitions for the v eviction add
        bvrow = cst.tile([1, CC], F32)
        nc.gpsimd.dma_start(bvrow[:], t["bv_d"][None, :])
        bvb = cst.tile([P, CC], F32)
        nc.gpsimd.partition_broadcast(bvb[:], bvrow[:])
        # b1 as per-partition [128, KF] for the gelu bias operand
        b1p = cst.tile([P, KF], F32)
        nc.gpsimd.dma_start(b1p[:], t["b1_d"].rearrange("(k p) -> p k", p=P))
        bo = cst.tile([1, D], F32R)
        b2 = cst.tile([1, D], F32R)
        nc.sync.dma_start(bo[:], t["bo_d"][None, :])
        nc.sync.dma_start(b2[:], t["b2_d"][None, :])

        # ---------------- helpers ----------------
        def layernorm(src_tiles, dst_tiles, sc):
            # var = E[x^2] - mu^2 (safe: |mu| << std for this data), so the
            # normalize is a single fused (x - mu) * inv DVE pass.
            for mt in range(NT):
                xt = src_tiles[mt]
                mu = sc.tile([P, 1], F32, tag="mu", name="mu")
                nc.vector.tensor_reduce(out=mu[:], in_=xt[:], op=ALU.add,
                                        axis=mybir.AxisListType.X)
                mus = sc.tile([P, 1], F32, tag="mus", name="mus")
                nc.scalar.mul(mus[:], mu[:], 1.0 / D)
                sq = sc.tile([P, D], F32, tag="sq", name="sq")
                sumsq = sc.tile([P, 1], F32, tag="sumsq", name="sumsq")
                nc.scalar.activation(sq[:], xt[:], AF.Square, accum_out=sumsq[:])
                mu2 = sc.tile([P, 1], F32, tag="mu2", name="mu2")
                nc.scalar.activation(mu2[:], mus[:], AF.Square)
                vpe = sc.tile([P, 1], F32, tag="vpe", name="vpe")
                # vpe = sumsq/D - mu2 + eps  (two tiny fused scalar ops)
                nc.vector.tensor_scalar(out=vpe[:], in0=sumsq[:],
                                        scalar1=1.0 / D, scalar2=mu2[:],
                                        op0=ALU.mult, op1=ALU.subtract)
                std = sc.tile([P, 1], F32, tag="std", name="std")
                nc.scalar.activation(std[:], vpe[:], AF.Sqrt, bias=epsc[:])
                inv = sc.tile([P, 1], F32, tag="inv", name="inv")
                nc.vector.reciprocal(inv[:], std[:])
                nc.vector.tensor_scalar(out=dst_tiles[mt][:], in0=xt[:],
                                        scalar1=mus[:], scalar2=inv[:],
                                        op0=ALU.subtract, op1=ALU.mult)

        def transpose_apply(src_tiles, dst_tiles, g_sb, b_sb, tp):
            for k in range(KD):
                for mt in range(NT):
                    ps = tp.tile([P, P], F32, tag="tps", name="tps")
                    nc.tensor.transpose(ps[:], src_tiles[mt][:, k * P:(k + 1) * P],
                                        ident[:])
                    nc.vector.tensor_scalar(
                        out=dst_tiles[k][:, mt * P:(mt + 1) * P], in0=ps[:],
                        scalar1=g_sb[:, k:k + 1], scalar2=b_sb[:, k:k + 1],
                        op0=ALU.mult, op1=ALU.add)

        # ============ phase A: LN1 + transpose + AllGather ============
        if True:
            with tc.tile_pool(name="lnA", bufs=1) as sc, \
                 tc.tile_pool(name="tpA", bufs=4, space="PSUM") as tp:
                h = [sc.tile([P, D], F32, tag=f"h{mt}", name=f"h{mt}") for mt in range(NT)]
                layernorm(xs, h, sc)
                hT = [sc.tile([P, TS], F32R, tag=f"hT{k}", name=f"hT{k}") for k in range(KD)]
                transpose_apply(h, hT, ln1g, ln1b, tp)
                for k in range(KD):
                    nc.sync.dma_start(t["ag1_in"][k * P:(k + 1) * P, :], hT[k][:])

            # ============ phases B+C: QKV + attention ============
            with tc.tile_pool(name="qkvP", bufs=1) as qkv:
                qT = [qkv.tile([P, S], F32R, tag=f"qT{m}", name=f"qT{m}") for m in range(2)]
                kT = [qkv.tile([P, S], F32R, tag=f"kT{m}", name=f"kT{m}") for m in range(2)]
                vo = [qkv.tile([P, HC, DH + 1], F32R, tag=f"vo{tm}", name=f"vo{tm}")
                      for tm in range(S // P)]

                wp_cm = tc.tile_pool(name="wqkv", bufs=1)
                wp = wp_cm.__enter__()
                # weight loads traced before the collective: DMA covers AG1
                wq = [wp.tile([P, CC], F32R, tag=f"wq{k}", name=f"wq{k}") for k in range(KD)]
                wk = [wp.tile([P, CC], F32R, tag=f"wk{k}", name=f"wk{k}") for k in range(KD)]
                wv = [wp.tile([P, CC], F32R, tag=f"wv{k}", name=f"wv{k}") for k in range(KD)]
                for k in range(KD):
                    nc.sync.dma_start(wq[k][:], t["wq_d"][k * P:(k + 1) * P, :])
                    nc.sync.dma_start(wk[k][:], t["wk_d"][k * P:(k + 1) * P, :])
                    nc.sync.dma_start(wv[k][:], t["wv_d"][k * P:(k + 1) * P, :])

                nc.gpsimd.collective_compute(
                    "AllGather", ALU.bypass, ins=[t["ag1_in"][:]],
                    outs=[t["ag1_out"][:]], replica_groups=GROUPS,
                )

                with tc.tile_pool(name="hTst", bufs=2) as st, \
                     tc.tile_pool(name="projPS", bufs=4, space="PSUM") as pps:
                    for qc in range(G):
                        hTq = [st.tile([P, TS], F32R, tag=f"hTq{k}", name=f"hTq{k}")
                               for k in range(KD)]
                        for k in range(KD):
                            nc.sync.dma_start(
                                hTq[k][:], t["ag1_out"][qc, k * P:(k + 1) * P, :])
                        for (w_sb, b_sb, out_sb) in ((wq, bqp, qT), (wk, bkp, kT)):
                            for m in range(2):
                                ps = pps.tile([P, TS], F32, tag="pps", name="pps")
                                for k in range(KD):
                                    nc.tensor.matmul(
                                        ps[:], w_sb[k][:, m * P:(m + 1) * P],
                                        hTq[k][:], start=(k == 0),
                                        stop=(k == KD - 1))
                                nc.vector.tensor_scalar(
                                    out=out_sb[m][:, qc * TS:(qc + 1) * TS],
                                    in0=ps[:], scalar1=b_sb[:, m:m + 1],
                                    scalar2=None, op0=ALU.add)
                        for mt in range(NT):
                            tm = qc * NT + mt
                            ps = pps.tile([P, CC], F32, tag="vps", name="vps")
                            for k in range(KD):
                                nc.tensor.matmul(
                                    ps[:], hTq[k][:, mt * P:(mt + 1) * P],
                                    wv[k][:], start=(k == 0), stop=(k == KD - 1))
                            nc.vector.tensor_tensor(
                                out=vo[tm][:, :, 0:DH],
                                in0=ps[:].rearrange("p (h e) -> p h e", h=HC),
                                in1=bvb[:].rearrange("p (h e) -> p h e", h=HC),
                                op=ALU.add)
                            nc.vector.tensor_copy(vo[tm][:, :, DH:DH + 1],
                                                  onescol4[:])

                wp_cm.__exit__(None, None, None)

                # ---- attention ----
                with (
                    tc.tile_pool(name="scPS", bufs=2, space="PSUM") as scp,
                    tc.tile_pool(name="avPS", bufs=2, space="PSUM") as avp,
                    tc.tile_pool(name="attnSB", bufs=3) as asb,
                ):
                    for hp in range(HC // 2):      # head pairs at PE rows 0/64
                        for qc in range(G):
                            kt_max = 4 * qc + 3
                            avs = [avp.tile([DH + 1, TS], F32, tag=f"av{j}",
                                            name=f"av{j}") for j in range(2)]
                            for kt in range(kt_max + 1):
                                # diag blocks: only columns >= v0 are ever read
                                w0 = P * max(0, kt - 4 * qc)
                                # both heads' score blocks into one 2-bank tile
                                sc_ps = scp.tile([P, 2, TS], F32,
                                                 tag="scp", name="scp")
                                for j in range(2):
                                    h_i = 2 * hp + j
                                    m = h_i // 2
                                    o = (h_i % 2) * DH
                                    nc.tensor.matmul(
                                        sc_ps[:, j, w0:],
                                        kT[m][o:o + DH, kt * P:(kt + 1) * P],
                                        qT[m][o:o + DH,
                                              qc * TS + w0:(qc + 1) * TS],
                                        start=True, stop=True)
                                e_r = asb.tile([P, 2, TS], F32R,
                                               tag="erp", name="erp")
                                if kt < 4 * qc:
                                    v0 = 0      # valid columns start
                                    nc.scalar.activation(
                                        e_r[:].rearrange("p a b -> p (a b)"),
                                        sc_ps[:].rearrange("p a b -> p (a b)"),
                                        AF.Exp, scale=0.125)
                                else:
                                    # diag block, shift s=-128*d: cols < 128*d
                                    # are fully masked -- never compute/read them
                                    d = kt - 4 * qc
                                    v0 = P * d
                                    e_f = asb.tile([P, 2, TS], F32,
                                                   tag="efp", name="efp")
                                    nc.scalar.activation(
                                        e_f[:, :, v0:], sc_ps[:, :, v0:],
                                        AF.Exp, scale=0.125)
                                    sh = 512 * qc - 128 * kt
                                    mdv = maskd[sh][:].rearrange(
                                        "p (a b) -> p a b", a=2)
                                    nc.vector.tensor_tensor(
                                        out=e_r[:, :, v0:],
                                        in0=e_f[:, :, v0:],
                                        in1=mdv[:, :, v0:],
                                        op=ALU.mult)
                                for j in range(2):
                                    h_i = 2 * hp + j
                                    nc.tensor.matmul(avs[j][:, v0:],
                                                     vo[kt][:, h_i, :],
                                                     e_r[:, j, v0:],
                                                     start=(kt == 0),
                                                     stop=(kt == kt_max))
                            for j in range(2):
                                h_i = 2 * hp + j
                                un = asb.tile([DH + 1, TS], F32,
                                              tag=f"un{j}", name=f"un{j}")
                                nc.vector.tensor_copy(un[:], avs[j][:])
                                rec = asb.tile([1, TS], F32,
                                               tag=f"rec{j}", name=f"rec{j}")
                                nc.vector.reciprocal(rec[:], un[DH:DH + 1, :])
                                rb = asb.tile([DH, TS], F32,
                                              tag=f"rb{j}", name=f"rb{j}")
                                nc.gpsimd.partition_broadcast(rb[:], rec[:])
                                chunk = asb.tile([DH, TS], F32R,
                                                 tag=f"chunk{j}", name=f"chunk{j}")
                                nc.vector.tensor_tensor(out=chunk[:],
                                                        in0=un[0:DH, :],
                                                        in1=rb[:], op=ALU.mult)
                                nc.sync.dma_start(
                                    t["a2a_in"][h_i * DH:(h_i + 1) * DH,
                                                qc * TS:(qc + 1) * TS],
                                    chunk[:])

            # prefetch pools traced before AG2 so DMA covers the collective
            pfw1_cm = tc.tile_pool(name="pfW1", bufs=1)
            pfw1 = pfw1_cm.__enter__()
            w1s0 = [pfw1.tile([P, 8 * P], F32R, tag=f"w1s{k}", name=f"w1s{k}")
                    for k in range(KD)]
            pfd_cm = tc.tile_pool(name="pfD", bufs=1)
            pfd = pfd_cm.__enter__()
            wo = [pfd.tile([P, D], F32R, tag=f"wo{k}", name=f"wo{k}") for k in range(KD)]
            for k in range(KD):
                nc.sync.dma_start(wo[k][:], t["wo_d"][k * P:(k + 1) * P, :])
                nc.sync.dma_start(w1s0[k][:], t["w1_d"][k * P:(k + 1) * P, 0:8 * P])

            nc.gpsimd.collective_compute(
                "AllGather", ALU.bypass, ins=[t["a2a_in"][:]],
                outs=[t["a2a_out"][:]], replica_groups=GROUPS,
            )

            # ============ phase D: out_proj + residual (in-place on xs) ====
            with tc.tile_pool(name="opPS", bufs=4, space="PSUM") as opp, \
                 tc.tile_pool(name="opSB", bufs=1) as osb:
                aT = [osb.tile([P, TS], F32R, tag=f"aT{k}", name=f"aT{k}") for k in range(KD)]
                off_sb = osb.tile([1, 1], mybir.dt.int32, tag="off", name="off")
                nc.sync.dma_start(off_sb[:], t["coff_d"][:])
                with nc.gpsimd.register("roff") as roff:
                    nc.gpsimd.reg_load(roff, off_sb[0:1, 0:1])
                    rv = nc.snap(roff)
                    for k in range(KD):
                        nc.gpsimd.dma_start(
                            aT[k][:],
                            t["a2a_out"][k // 2, (k % 2) * P:(k % 2 + 1) * P,
                                         bass.ds(rv, TS)])
                for mt in range(NT):
                    for n in range(2):
                        ps = opp.tile([P, TS], F32, tag="op", name="op")
                        nc.tensor.matmul(ps[:], ones128[:],
                                         bo[:, n * TS:(n + 1) * TS],
                                         start=True, stop=False)
                        for k in range(KD):
                            nc.tensor.matmul(
                                ps[:], aT[k][:, mt * P:(mt + 1) * P],
                                wo[k][:, n * TS:(n + 1) * TS],
                                start=False, stop=(k == KD - 1))
                        # residual written in place: xs becomes x2
                        nc.vector.tensor_tensor(
                            out=xs[mt][:, n * TS:(n + 1) * TS], in0=ps[:],
                            in1=xs[mt][:, n * TS:(n + 1) * TS], op=ALU.add)
            pfd_cm.__exit__(None, None, None)
            x2 = xs

            with tc.tile_pool(name="h2TP", bufs=1) as h2tp:
                h2T = [h2tp.tile([P, TS], F32R, tag=f"h2T{k}", name=f"h2T{k}")
                       for k in range(KD)]
                with tc.tile_pool(name="lnD", bufs=1) as sc, \
                     tc.tile_pool(name="tpD", bufs=4, space="PSUM") as tp:
                    h2 = [sc.tile([P, D], F32, tag=f"h2{mt}", name=f"h2{mt}")
                          for mt in range(NT)]
                    layernorm(x2, h2, sc)
                    transpose_apply(h2, h2T, ln2g, ln2b, tp)

                # ============ phase E: FFN ============
                with tc.tile_pool(name="gTP", bufs=1) as gtp:
                    gT = [gtp.tile([P, TS], F32R, tag=f"gT{mf}", name=f"gT{mf}")
                          for mf in range(KF)]
                    MFB = 4     # mf tiles per w1 stream block
                    with tc.tile_pool(name="w1st", bufs=1) as w1p, \
                         tc.tile_pool(name="gPS", bufs=4, space="PSUM") as gps:
                        for blk in range(KF // MFB):
                            if blk < 2:
                                # prefetched during AG2 (w1s0 holds blocks 0-1)
                                w1s = [w1s0[k][:, blk * MFB * P:(blk + 1) * MFB * P]
                                       for k in range(KD)]
                            else:
                                w1t = [w1p.tile([P, MFB * P], F32R,
                                                tag=f"w1b{k}", name=f"w1b{k}",
                                                bufs=2)
                                       for k in range(KD)]
                                for k in range(KD):
                                    nc.sync.dma_start(
                                        w1t[k][:],
                                        t["w1_d"][k * P:(k + 1) * P,
                                                  blk * MFB * P:(blk + 1) * MFB * P])
                                w1s = [w1t[k][:] for k in range(KD)]
                            for j in range(MFB):
                                mf = blk * MFB + j
                                ps = gps.tile([P, TS], F32, tag="g", name="g")
                                for k in range(KD):
                                    nc.tensor.matmul(
                                        ps[:], w1s[k][:, j * P:(j + 1) * P],
                                        h2T[k][:], start=(k == 0),
                                        stop=(k == KD - 1))
                                nc.scalar.activation(gT[mf][:], ps[:],
                                                     AF.Gelu,
                                                     bias=b1p[:, mf:mf + 1])

                    with tc.tile_pool(name="w2st", bufs=4) as w2p, \
                         tc.tile_pool(name="fPS", bufs=1, space="PSUM") as fps, \
                         tc.tile_pool(name="ySB", bufs=2) as ysb:
                        f_ps = [fps.tile([P, D], F32, tag=f"f{mt}", name=f"f{mt}")
                                for mt in range(NT)]
                        for mt in range(NT):
                            for n in range(2):
                                nc.tensor.matmul(
                                    f_ps[mt][:, n * TS:(n + 1) * TS],
                                    ones128[:], b2[:, n * TS:(n + 1) * TS],
                                    start=True, stop=False)
                        for k2 in range(KF):
                            w2t = w2p.tile([P, D], F32R, tag="w2", name="w2")
                            nc.sync.dma_start(
                                w2t[:], t["w2_d"][k2 * P:(k2 + 1) * P, :])
                            for mt in range(NT):
                                for n in range(2):
                                    nc.tensor.matmul(
                                        f_ps[mt][:, n * TS:(n + 1) * TS],
                                        gT[k2][:, mt * P:(mt + 1) * P],
                                        w2t[:, n * TS:(n + 1) * TS],
                                        start=False, stop=(k2 == KF - 1))
                        for mt in range(NT):
                            yt = ysb.tile([P, D], F32, tag="y", name="y")
                            nc.vector.tensor_tensor(out=yt[:],
                                                    in0=f_ps[mt][:],
                                                    in1=x2[mt][:],
                                                    op=ALU.add)
                            nc.sync.dma_start(
                                t["y_d"][mt * P:(mt + 1) * P, :], yt[:])
            pfw1_cm.__exit__(None, None, None)
            xsp_cm.__exit__(None, None, None)


def _in_maps(inputs):
    f32 = np.float32
    maps = []
    for c in range(NC):
        b, r = c // G, c % G
        c0 = r * CC
        m = {
            "x": np.ascontiguousarray(np.asarray(inputs["x"])[b, r * TS:(r + 1) * TS, :], f32),
            "ln1_g": np.ascontiguousarray(inputs["ln1_g"], f32),
            "ln1_b": np.ascontiguousarray(inputs["ln1_b"], f32),
            "Wq": np.ascontiguousarray(np.asarray(inputs["Wq"])[:, c0:c0 + CC], f32),
            "Wk": np.ascontiguousarray(np.asarray(inputs["Wk"])[:, c0:c0 + CC], f32),
            "Wv": np.ascontiguousarray(np.asarray(inputs["Wv"])[:, c0:c0 + CC], f32),
            "bq": np.ascontiguousarray(np.asarray(inputs["bq"])[c0:c0 + CC], f32),
            "bk": np.ascontiguousarray(np.asarray(inputs["bk"])[c0:c0 + CC], f32),
            "bv": np.ascontiguousarray(np.asarray(inputs["bv"])[c0:c0 + CC], f32),
            "Wo": np.ascontiguousarray(inputs["Wo"], f32),
            "bo": np.ascontiguousarray(inputs["bo"], f32),
            "ln2_g": np.ascontiguousarray(inputs["ln2_g"], f32),
            "ln2_b": np.ascontiguousarray(inputs["ln2_b"], f32),
            "W1": np.ascontiguousarray(inputs["W1"], f32),
            "b1": np.ascontiguousarray(inputs["b1"], f32),
            "W2": np.ascontiguousarray(inputs["W2"], f32),
            "b2": np.ascontiguousarray(inputs["b2"], f32),
            "coff": np.array([[r * TS]], dtype=np.int32),
        }
        maps.append(m)
    return maps


def _run(inputs, trace=False):
    if "nc" not in _CACHE:
        _CACHE["nc"] = build()
    nc = _CACHE["nc"]
    maps = _in_maps(inputs)
    res = run_bass_kernel_spmd(nc, maps, list(range(NC)), trace=trace)
    out = np.empty((B, S, D), np.float32)
    for c in range(NC):
        b, r = c // G, c % G
        out[b, r * TS:(r + 1) * TS, :] = res.results[c]["y"]
    return out, res


def kernel(**inputs):
    out, _ = _run(inputs, trace=False)
    return out


if __name__ == "__main__":
    build()
    print("build OK")



# revision 9
# speedup vs baseline: 1.8238x; 1.8238x over previous
"""Trainium2 Bass kernel for a dense transformer block (B=2,S=2048,D=1024,H=16,DFF=4096).

Sharding across 8 NeuronCores:
  core c: batch b=c//4, group rank r=c%4, replica groups [[0,1,2,3],[4,5,6,7]].
  - Every core loads the FULL x of its batch; LN1 + transpose are replicated
    (no collective needed for the attention input).
  - Attention: head-parallel (4 heads/core, full causal sequence), output
    kept on-chip.
  - out_proj: each core computes the partial sum over its own heads for ALL
    2048 tokens; a single ReduceScatter(add) in bf16 then hands each core its
    own summed 512-token strip (output bytes 1MB vs the 8MB an AllGather of
    the attention output would move).
  - residual, LN2, FFN: token-sharded (512 tokens/core), full weights.
Matmuls run in bf16 (psum accumulation stays fp32); LN statistics and the
residual spine stay fp32.
"""
import sys

sys.path.insert(0, "/opt/trn_rl_repo")

import numpy as np
import ml_dtypes

import concourse.bass as bass
import concourse.mybir as mybir
import concourse.tile as tile
from concourse import bacc
from concourse.bass_utils import run_bass_kernel_spmd
from concourse.masks import make_identity

AF = mybir.ActivationFunctionType
ALU = mybir.AluOpType
F32 = mybir.dt.float32
F32R = mybir.dt.float32r
BF16 = mybir.dt.bfloat16

B, S, D, H = 2, 2048, 1024, 16
DH = D // H          # 64
DFF = 4 * D          # 4096
EPS = 1e-5
NC = 8               # cores
G = 4                # cores per group (per batch)
TS = S // G          # 512 tokens per strip
HC = H // G          # 4 heads per core
CC = HC * DH         # 256 head-columns per core
P = 128
KD = D // P          # 8 k-tiles over D
KF = DFF // P        # 32 k-tiles over DFF
NTT = S // P         # 16 token tiles (full sequence)
NT = TS // P         # 4 token tiles per strip
GROUPS = [[0, 1, 2, 3], [4, 5, 6, 7]]

_CACHE = {}


def build():
    nc = bacc.Bacc(None)

    io = {}
    io["x_d"] = nc.declare_dram_parameter("x", [S, D], F32, isOutput=False)
    io["xo_d"] = nc.declare_dram_parameter("xo", [TS, D], F32, isOutput=False)
    io["ln1g_d"] = nc.declare_dram_parameter("ln1_g", [D], F32, isOutput=False)
    io["ln1b_d"] = nc.declare_dram_parameter("ln1_b", [D], F32, isOutput=False)
    io["wq_d"] = nc.declare_dram_parameter("Wq", [D, CC], BF16, isOutput=False)
    io["wk_d"] = nc.declare_dram_parameter("Wk", [D, CC], BF16, isOutput=False)
    io["wv_d"] = nc.declare_dram_parameter("Wv", [D, CC], BF16, isOutput=False)
    io["bq_d"] = nc.declare_dram_parameter("bq", [CC], F32, isOutput=False)
    io["bk_d"] = nc.declare_dram_parameter("bk", [CC], F32, isOutput=False)
    io["bv_d"] = nc.declare_dram_parameter("bv", [CC], F32, isOutput=False)
    io["wo_d"] = nc.declare_dram_parameter("Wo", [CC, D], BF16, isOutput=False)
    io["bo_d"] = nc.declare_dram_parameter("bo", [D], F32, isOutput=False)
    io["ln2g_d"] = nc.declare_dram_parameter("ln2_g", [D], F32, isOutput=False)
    io["ln2b_d"] = nc.declare_dram_parameter("ln2_b", [D], F32, isOutput=False)
    io["w1_d"] = nc.declare_dram_parameter("W1", [D, DFF], BF16, isOutput=False)
    io["b1_d"] = nc.declare_dram_parameter("b1", [DFF], F32, isOutput=False)
    io["w2_d"] = nc.declare_dram_parameter("W2", [DFF, D], BF16, isOutput=False)
    io["b2_d"] = nc.declare_dram_parameter("b2", [D], F32R, isOutput=False)
    io["y_d"] = nc.declare_dram_parameter("y", [TS, D], F32, isOutput=True)

    io["rs_in"] = nc.dram_tensor("rs_in", [S, D], BF16)
    io["rs_out"] = nc.dram_tensor("rs_out", [TS, D], BF16)

    with tile.TileContext(nc) as tc:
        _body(nc, tc, io)
    nc.compile()
    return nc


def _body(nc, tc, t):
    with tc.tile_pool(name="const", bufs=1) as cst:
        # ---- tiny constants + the LN1 params gate phase A: issue first ----
        ln1g = cst.tile([P, KD], F32)
        ln1b = cst.tile([P, KD], F32)
        nc.sync.dma_start(ln1g[:], t["ln1g_d"].rearrange("(k p) -> p k", p=P))
        nc.sync.dma_start(ln1b[:], t["ln1b_d"].rearrange("(k p) -> p k", p=P))

        identb = cst.tile([P, P], BF16)
        make_identity(nc, identb[:])
        epsc = cst.tile([P, 1], F32)
        nc.gpsimd.memset(epsc[:], EPS)
        onesrow_f = cst.tile([1, P], F32)
        nc.gpsimd.memset(onesrow_f[:], 1.0)
        ones128 = cst.tile([1, P], F32R)
        nc.vector.tensor_copy(ones128[:], onesrow_f[:])
        onescol4 = cst.tile([P, HC, 1], F32)
        nc.gpsimd.memset(onescol4[:], 1.0)

        # ---------------- helpers ----------------
        def layernorm(src_tiles, dst_tiles, sc, sq):
            # var = E[x^2] - mu^2; normalize is one fused (x - mu) * inv pass.
            # reduce/normalize alternate between DVE and Pool to halve the
            # DVE serial time; the Square/Sqrt chain stays on Act.
            n = len(src_tiles)
            for mt in range(n):
                ve = nc.vector if mt % 2 == 0 else nc.gpsimd
                xt = src_tiles[mt]
                mu = sc.tile([P, 1], F32, tag="mu", name="mu")
                nc.vector.tensor_reduce(out=mu[:], in_=xt[:], op=ALU.add,
                                        axis=mybir.AxisListType.X)
                mus = sc.tile([P, 1], F32, tag="mus", name="mus")
                nc.scalar.mul(mus[:], mu[:], 1.0 / D)
                sumsq = sc.tile([P, 1], F32, tag="sumsq", name="sumsq")
                nc.scalar.activation(sq[:], xt[:], AF.Square, accum_out=sumsq[:])
                mu2 = sc.tile([P, 1], F32, tag="mu2", name="mu2")
                nc.scalar.activation(mu2[:], mus[:], AF.Square)
                vpe = sc.tile([P, 1], F32, tag="vpe", name="vpe")
                ve.tensor_scalar(out=vpe[:], in0=sumsq[:],
                                 scalar1=1.0 / D, scalar2=mu2[:],
                                 op0=ALU.mult, op1=ALU.subtract)
                std = sc.tile([P, 1], F32, tag="std", name="std")
                nc.scalar.activation(std[:], vpe[:], AF.Sqrt, bias=epsc[:])
                inv = sc.tile([P, 1], F32, tag="inv", name="inv")
                nc.vector.reciprocal(inv[:], std[:])
                ve.tensor_scalar(out=dst_tiles[mt][:], in0=xt[:],
                                 scalar1=mus[:], scalar2=inv[:],
                                 op0=ALU.subtract, op1=ALU.mult)

        def transpose_strip(h_tiles, dst, g_sb, b_sb, tp, k, base_mt, nmt,
                            evict_act):
            # transpose nmt 128x128 blocks of column-tile k into one psum
            # bank, then evict once with the fused *g+b (DVE or Act).
            ps = tp.tile([P, NT * P], BF16, tag="tps", name="tps")
            for j in range(nmt):
                nc.tensor.transpose(ps[:, j * P:(j + 1) * P],
                                    h_tiles[base_mt + j][:, k * P:(k + 1) * P],
                                    identb[:])
            w = nmt * P
            if evict_act:
                nc.scalar.activation(dst[:, :w], ps[:, :w], AF.Identity,
                                     bias=b_sb[:, k:k + 1],
                                     scale=g_sb[:, k:k + 1])
            else:
                nc.vector.tensor_scalar(out=dst[:, :w], in0=ps[:, :w],
                                        scalar1=g_sb[:, k:k + 1],
                                        scalar2=b_sb[:, k:k + 1],
                                        op0=ALU.mult, op1=ALU.add)

        # wo lives in the const pool (tiny, loaded at C-start)
        wo = [cst.tile([P, D], BF16, tag=f"wo{kk}", name=f"wo{kk}")
              for kk in range(2)]

        # ============ persistent pools (LIFO lifetimes) ============
        xop_cm = tc.tile_pool(name="xop", bufs=1)
        xop = xop_cm.__enter__()
        xo = [xop.tile([P, D], F32, tag=f"xo{mt}", name=f"xo{mt}")
              for mt in range(NT)]

        w1p_cm = tc.tile_pool(name="w1p", bufs=1)
        w1p = w1p_cm.__enter__()
        w1s = [w1p.tile([P, DFF], BF16, tag=f"w1s{k}", name=f"w1s{k}")
               for k in range(KD)]

        qkvp_cm = tc.tile_pool(name="qkvP", bufs=1)
        qkv = qkvp_cm.__enter__()
        qT = [qkv.tile([P, S], BF16, tag=f"qT{m}", name=f"qT{m}") for m in range(2)]
        kT = [qkv.tile([P, S], BF16, tag=f"kT{m}", name=f"kT{m}") for m in range(2)]
        vo = [qkv.tile([P, HC, DH + 1], BF16, tag=f"vo{tm}", name=f"vo{tm}")
              for tm in range(NTT)]
        aT = [qkv.tile([P, S], BF16, tag=f"aT{kk}", name=f"aT{kk}")
              for kk in range(2)]

        wqkvp_cm = tc.tile_pool(name="wqkv", bufs=1)
        wqkvp = wqkvp_cm.__enter__()
        wq = [wqkvp.tile([P, CC], BF16, tag=f"wq{k}", name=f"wq{k}") for k in range(KD)]
        wk = [wqkvp.tile([P, CC], BF16, tag=f"wk{k}", name=f"wk{k}") for k in range(KD)]
        wv = [wqkvp.tile([P, CC], BF16, tag=f"wv{k}", name=f"wv{k}") for k in range(KD)]

        hTp_cm = tc.tile_pool(name="hTp", bufs=1)
        hTp = hTp_cm.__enter__()
        hT = [hTp.tile([P, S], BF16, tag=f"hT{k}", name=f"hT{k}")
              for k in range(KD)]

        # ============ phase A: x stream + LN1 + transpose ============
        with tc.tile_pool(name="xs", bufs=2) as xsp, \
             tc.tile_pool(name="hs", bufs=1) as hsp, \
             tc.tile_pool(name="sqA", bufs=1) as sqp, \
             tc.tile_pool(name="lnA", bufs=2) as sc, \
             tc.tile_pool(name="tpA", bufs=2, space="PSUM") as tpA:
            sqA = sqp.tile([P, D], F32, tag="sqA", name="sqA")
            for mtg in range(NTT // NT):
                hcur = []
                for j in range(NT):
                    mt = mtg * NT + j
                    xt = xsp.tile([P, D], F32, tag="x", name="x")
                    nc.sync.dma_start(xt[:], t["x_d"][mt * P:(mt + 1) * P, :])
                    ht = hsp.tile([P, D], BF16, tag=f"h{j}", name=f"h{j}")
                    layernorm([xt], [ht], sc, sqA)
                    hcur.append(ht)
                for k in range(KD):
                    transpose_strip(hcur,
                                    hT[k][:, mtg * NT * P:(mtg + 1) * NT * P],
                                    ln1g, ln1b, tpA, k, 0, NT,
                                    evict_act=(k % 2 == 1))
                if mtg == 0:
                    # own-strip x + weight prefetch, queued behind the first
                    # tile group so they don't starve the x stream
                    for mt in range(NT):
                        nc.sync.dma_start(xo[mt][:],
                                          t["xo_d"][mt * P:(mt + 1) * P, :])
                    for k in range(KD):
                        nc.gpsimd.dma_start(wq[k][:], t["wq_d"][k * P:(k + 1) * P, :])
                        nc.gpsimd.dma_start(wk[k][:], t["wk_d"][k * P:(k + 1) * P, :])
                        nc.gpsimd.dma_start(wv[k][:], t["wv_d"][k * P:(k + 1) * P, :])

        # remaining small constants (issued behind x on SP; needed later)
        ln2g = cst.tile([P, KD], F32)
        ln2b = cst.tile([P, KD], F32)
        nc.sync.dma_start(ln2g[:], t["ln2g_d"].rearrange("(k p) -> p k", p=P))
        nc.sync.dma_start(ln2b[:], t["ln2b_d"].rearrange("(k p) -> p k", p=P))
        bqp = cst.tile([P, 2], F32)
        bkp = cst.tile([P, 2], F32)
        nc.sync.dma_start(bqp[:], t["bq_d"].rearrange("(m p) -> p m", p=P))
        nc.sync.dma_start(bkp[:], t["bk_d"].rearrange("(m p) -> p m", p=P))
        bvrow = cst.tile([1, CC], F32)
        nc.sync.dma_start(bvrow[:], t["bv_d"][None, :])
        bvb = cst.tile([P, CC], F32)
        nc.gpsimd.partition_broadcast(bvb[:], bvrow[:])
        borow = cst.tile([1, D], F32)
        nc.sync.dma_start(borow[:], t["bo_d"][None, :])
        bob = cst.tile([P, D], F32)
        nc.gpsimd.partition_broadcast(bob[:], borow[:])
        b1p = cst.tile([P, KF], F32)
        nc.sync.dma_start(b1p[:], t["b1_d"].rearrange("(k p) -> p k", p=P))
        b2r = cst.tile([1, D], F32R)
        nc.sync.dma_start(b2r[:], t["b2_d"][None, :])

        # doubled causal masks (mask||mask so one DVE op masks a 2-head pair)
        maskd = {}
        for sh in (0, -128, -256, -384):
            md = cst.tile([P, 2 * TS], BF16, tag=f"maskd{sh}", name=f"maskd{sh}")
            nc.gpsimd.memset(md[:], 1.0)
            for half in range(2):
                nc.gpsimd.affine_select(
                    out=md[:, half * TS:(half + 1) * TS],
                    in_=md[:, half * TS:(half + 1) * TS],
                    compare_op=ALU.is_ge, fill=0.0, base=sh,
                    pattern=[[1, TS]], channel_multiplier=-1,
                )
            maskd[sh] = md

        # ============ phase B: QKV projections ============
        with tc.tile_pool(name="projPS", bufs=2, space="PSUM") as pps, \
             tc.tile_pool(name="vPS", bufs=2, space="PSUM") as vps:
            for (w_sb, b_sb, out_sb) in ((wq, bqp, qT), (wk, bkp, kT)):
                for m in range(2):
                    for st in range(G):
                        ps = pps.tile([P, TS], F32, tag="pp", name="pp")
                        for k in range(KD):
                            nc.tensor.matmul(
                                ps[:], w_sb[k][:, m * P:(m + 1) * P],
                                hT[k][:, st * TS:(st + 1) * TS],
                                start=(k == 0), stop=(k == KD - 1))
                        if st % 2 == 0:
                            nc.vector.tensor_scalar(
                                out=out_sb[m][:, st * TS:(st + 1) * TS],
                                in0=ps[:], scalar1=b_sb[:, m:m + 1],
                                scalar2=None, op0=ALU.add)
                        else:
                            nc.scalar.activation(
                                out_sb[m][:, st * TS:(st + 1) * TS], ps[:],
                                AF.Identity, bias=b_sb[:, m:m + 1])
            for tm in range(NTT):
                ps = vps.tile([P, CC], F32, tag="vp", name="vp")
                for k in range(KD):
                    nc.tensor.matmul(
                        ps[:], hT[k][:, tm * P:(tm + 1) * P], wv[k][:],
                        start=(k == 0), stop=(k == KD - 1))
                nc.vector.tensor_tensor(
                    out=vo[tm][:, :, 0:DH],
                    in0=ps[:].rearrange("p (h e) -> p h e", h=HC),
                    in1=bvb[:].rearrange("p (h e) -> p h e", h=HC),
                    op=ALU.add)
                nc.vector.tensor_copy(vo[tm][:, :, DH:DH + 1], onescol4[:])

        hTp_cm.__exit__(None, None, None)
        wqkvp_cm.__exit__(None, None, None)

        # W1 + Wo prefetch (DMA idles during attention; needed at D / fc1)
        for kk in range(2):
            nc.gpsimd.dma_start(wo[kk][:], t["wo_d"][kk * P:(kk + 1) * P, :])
        for k in range(KD):
            nc.gpsimd.dma_start(w1s[k][:], t["w1_d"][k * P:(k + 1) * P, :])

        # ============ phase C: attention (head-parallel, causal) ============
        with (
            tc.tile_pool(name="scPS", bufs=2, space="PSUM") as scp,
            tc.tile_pool(name="avPS", bufs=2, space="PSUM") as avp,
            tc.tile_pool(name="attnSB", bufs=3) as asb,
        ):
            for hp in range(HC // 2):      # head pairs at PE rows 0/64
                for qc in range(G):
                    kt_max = 4 * qc + 3
                    avs = [avp.tile([DH + 1, TS], F32, tag=f"av{j}",
                                    name=f"av{j}") for j in range(2)]
                    for kt in range(kt_max + 1):
                        # diag blocks: only columns >= v0 are ever read
                        w0 = P * max(0, kt - 4 * qc)
                        sc_ps = scp.tile([P, 2, TS], F32, tag="scp", name="scp")
                        for j in range(2):
                            o = j * DH
                            nc.tensor.matmul(
                                sc_ps[:, j, w0:],
                                kT[hp][o:o + DH, kt * P:(kt + 1) * P],
                                qT[hp][o:o + DH, qc * TS + w0:(qc + 1) * TS],
                                start=True, stop=True)
                        e_r = asb.tile([P, 2, TS], BF16, tag="erp", name="erp")
                        if kt < 4 * qc:
                            v0 = 0      # valid columns start
                            nc.scalar.activation(
                                e_r[:].rearrange("p a b -> p (a b)"),
                                sc_ps[:].rearrange("p a b -> p (a b)"),
                                AF.Exp, scale=0.125)
                        else:
                            # diag block, shift s=-128*d: cols < 128*d are
                            # fully masked -- never compute/read them
                            d = kt - 4 * qc
                            v0 = P * d
                            e_f = asb.tile([P, 2, TS], BF16, tag="efp", name="efp")
                            nc.scalar.activation(
                                e_f[:, :, v0:], sc_ps[:, :, v0:],
                                AF.Exp, scale=0.125)
                            sh = 512 * qc - 128 * kt
                            mdv = maskd[sh][:].rearrange("p (a b) -> p a b", a=2)
                            nc.vector.tensor_tensor(
                                out=e_r[:, :, v0:], in0=e_f[:, :, v0:],
                                in1=mdv[:, :, v0:], op=ALU.mult)
                        for j in range(2):
                            nc.tensor.matmul(avs[j][:, v0:],
                                             vo[kt][:, 2 * hp + j, :],
                                             e_r[:, j, v0:],
                                             start=(kt == 0),
                                             stop=(kt == kt_max))
                    for j in range(2):
                        rec = asb.tile([1, TS], F32, tag=f"rec{j}",
                                       name=f"rec{j}")
                        nc.vector.reciprocal(rec[:], avs[j][DH:DH + 1, :])
                        rb = asb.tile([DH, TS], F32, tag=f"rb{j}",
                                      name=f"rb{j}")
                        nc.gpsimd.partition_broadcast(rb[:], rec[:])
                        nc.vector.tensor_tensor(
                            out=aT[hp][j * DH:(j + 1) * DH,
                                       qc * TS:(qc + 1) * TS],
                            in0=avs[j][0:DH, :], in1=rb[:], op=ALU.mult)

        # ============ phase D: out_proj partials for ALL tokens ============
        with tc.tile_pool(name="opPS", bufs=2, space="PSUM") as opp, \
             tc.tile_pool(name="opSB", bufs=3) as osb:
            for mt in range(NTT):
                ps = opp.tile([P, 2, TS], F32, tag="op", name="op")
                for n in range(2):
                    for kk in range(2):
                        nc.tensor.matmul(
                            ps[:, n, :], aT[kk][:, mt * P:(mt + 1) * P],
                            wo[kk][:, n * TS:(n + 1) * TS],
                            start=(kk == 0), stop=(kk == 1))
                ot = osb.tile([P, D], BF16, tag="ot", name="ot")
                if mt % 2 == 0:
                    nc.vector.tensor_copy(
                        ot[:], ps[:].rearrange("p a b -> p (a b)"))
                else:
                    nc.scalar.activation(
                        ot[:], ps[:].rearrange("p a b -> p (a b)"), AF.Copy)
                nc.sync.dma_start(t["rs_in"][mt * P:(mt + 1) * P, :], ot[:])

        qkvp_cm.__exit__(None, None, None)

        # ============ ReduceScatter: sum head-partials, keep own strip ======
        nc.gpsimd.collective_compute(
            "ReduceScatter", ALU.add, ins=[t["rs_in"][:]],
            outs=[t["rs_out"][:]], replica_groups=GROUPS,
        )

        # ============ phase E: residual + LN2 + FFN (token-sharded) ========
        gTp_cm = tc.tile_pool(name="gTp", bufs=1)
        gtp = gTp_cm.__enter__()
        gT = [gtp.tile([P, TS], BF16, tag=f"gT{mf}", name=f"gT{mf}")
              for mf in range(KF)]
        h2Tp_cm = tc.tile_pool(name="h2Tp", bufs=1)
        h2tp = h2Tp_cm.__enter__()
        h2T = [h2tp.tile([P, TS], BF16, tag=f"h2T{k}", name=f"h2T{k}")
               for k in range(KD)]

        with tc.tile_pool(name="rsb", bufs=1) as rsb, \
             tc.tile_pool(name="lnD", bufs=2) as sc2, \
             tc.tile_pool(name="tpD", bufs=2, space="PSUM") as tpD:
            rst = [rsb.tile([P, D], BF16, tag=f"rs{mt}", name=f"rs{mt}")
                   for mt in range(NT)]
            for mt in range(NT):
                nc.sync.dma_start(rst[mt][:],
                                  t["rs_out"][mt * P:(mt + 1) * P, :])
            for mt in range(NT):
                ve = nc.vector if mt % 2 == 0 else nc.gpsimd
                ve.tensor_tensor(out=xo[mt][:], in0=xo[mt][:], in1=rst[mt][:],
                                 op=ALU.add)
                ve.tensor_tensor(out=xo[mt][:], in0=xo[mt][:], in1=bob[:],
                                 op=ALU.add)
            sqD = sc2.tile([P, D], F32, tag="sqD", name="sqD")
            h2 = [sc2.tile([P, D], BF16, tag=f"h2{mt}", name=f"h2{mt}")
                  for mt in range(NT)]
            layernorm(xo, h2, sc2, sqD)
            for k in range(KD):
                transpose_strip(h2, h2T[k][:], ln2g, ln2b, tpD, k, 0, NT,
                                evict_act=(k % 2 == 1))

        # ---- fc1 + gelu ----
        with tc.tile_pool(name="gPS", bufs=4, space="PSUM") as gps:
            for mf in range(KF):
                ps = gps.tile([P, TS], F32, tag="g", name="g")
                for k in range(KD):
                    nc.tensor.matmul(
                        ps[:], w1s[k][:, mf * P:(mf + 1) * P], h2T[k][:],
                        start=(k == 0), stop=(k == KD - 1))
                nc.scalar.activation(gT[mf][:], ps[:], AF.Gelu,
                                     bias=b1p[:, mf:mf + 1])
        h2Tp_cm.__exit__(None, None, None)

        # ---- fc2 + residual ----
        with tc.tile_pool(name="w2st", bufs=8) as w2p, \
             tc.tile_pool(name="fPS", bufs=1, space="PSUM") as fps, \
             tc.tile_pool(name="ySB", bufs=2) as ysb:
            f_ps = [fps.tile([P, 2, TS], F32, tag=f"f{mt}", name=f"f{mt}")
                    for mt in range(NT)]
            for mt in range(NT):
                for n in range(2):
                    nc.tensor.matmul(
                        f_ps[mt][:, n, :], ones128[:],
                        b2r[:, n * TS:(n + 1) * TS], start=True, stop=False)
            for k2 in range(KF):
                w2t = w2p.tile([P, D], BF16, tag="w2", name="w2")
                nc.scalar.dma_start(w2t[:], t["w2_d"][k2 * P:(k2 + 1) * P, :])
                for mt in range(NT):
                    for n in range(2):
                        nc.tensor.matmul(
                            f_ps[mt][:, n, :],
                            gT[k2][:, mt * P:(mt + 1) * P],
                            w2t[:, n * TS:(n + 1) * TS],
                            start=False, stop=(k2 == KF - 1))
            for mt in range(NT):
                yt = ysb.tile([P, D], F32, tag="y", name="y")
                nc.vector.tensor_tensor(
                    out=yt[:], in0=f_ps[mt][:].rearrange("p a b -> p (a b)"),
                    in1=xo[mt][:], op=ALU.add)
                nc.sync.dma_start(t["y_d"][mt * P:(mt + 1) * P, :], yt[:])
        gTp_cm.__exit__(None, None, None)
        w1p_cm.__exit__(None, None, None)
        xop_cm.__exit__(None, None, None)


def _in_maps(inputs):
    f32 = np.float32
    bf16 = ml_dtypes.bfloat16

    def as_bf16(a):
        return np.ascontiguousarray(np.asarray(a, f32).astype(bf16))

    x = np.asarray(inputs["x"], f32)
    maps = []
    for c in range(NC):
        b, r = c // G, c % G
        c0 = r * CC
        m = {
            "x": np.ascontiguousarray(x[b]),
            "xo": np.ascontiguousarray(x[b, r * TS:(r + 1) * TS, :]),
            "ln1_g": np.ascontiguousarray(inputs["ln1_g"], f32),
            "ln1_b": np.ascontiguousarray(inputs["ln1_b"], f32),
            "Wq": as_bf16(np.asarray(inputs["Wq"], f32)[:, c0:c0 + CC]),
            "Wk": as_bf16(np.asarray(inputs["Wk"], f32)[:, c0:c0 + CC]),
            "Wv": as_bf16(np.asarray(inputs["Wv"], f32)[:, c0:c0 + CC]),
            "bq": np.ascontiguousarray(np.asarray(inputs["bq"], f32)[c0:c0 + CC]),
            "bk": np.ascontiguousarray(np.asarray(inputs["bk"], f32)[c0:c0 + CC]),
            "bv": np.ascontiguousarray(np.asarray(inputs["bv"], f32)[c0:c0 + CC]),
            "Wo": as_bf16(np.asarray(inputs["Wo"], f32)[c0:c0 + CC, :]),
            "bo": np.ascontiguousarray(inputs["bo"], f32),
            "ln2_g": np.ascontiguousarray(inputs["ln2_g"], f32),
            "ln2_b": np.ascontiguousarray(inputs["ln2_b"], f32),
            "W1": as_bf16(inputs["W1"]),
            "b1": np.ascontiguousarray(inputs["b1"], f32),
            "W2": as_bf16(inputs["W2"]),
            "b2": np.ascontiguousarray(inputs["b2"], f32),
        }
        maps.append(m)
    return maps


def _run(inputs, trace=False):
    if "nc" not in _CACHE:
        _CACHE["nc"] = build()
    nc = _CACHE["nc"]
    maps = _in_maps(inputs)
    res = run_bass_kernel_spmd(nc, maps, list(range(NC)), trace=trace)
    out = np.empty((B, S, D), np.float32)
    for c in range(NC):
        b, r = c // G, c % G
        out[b, r * TS:(r + 1) * TS, :] = res.results[c]["y"]
    return out, res


def kernel(**inputs):
    out, _ = _run(inputs, trace=False)
    return out


if __name__ == "__main__":
    build()
    print("build OK")


# revision 13
# speedup vs baseline: 1.8361x; 1.0067x over previous
"""Trainium2 Bass kernel for a dense transformer block (B=2,S=2048,D=1024,H=16,DFF=4096).

Sharding across 8 NeuronCores:
  core c: batch b=c//4, group rank r=c%4, replica groups [[0,1,2,3],[4,5,6,7]].
  - Every core loads the FULL x of its batch; LN1 + transpose are replicated
    (no collective needed for the attention input).
  - Attention: head-parallel (4 heads/core, full causal sequence), output
    kept on-chip.
  - out_proj: each core computes the partial sum over its own heads for ALL
    2048 tokens; a single ReduceScatter(add) in bf16 then hands each core its
    own summed 512-token strip (output bytes 1MB vs the 8MB an AllGather of
    the attention output would move).
  - residual, LN2, FFN: token-sharded (512 tokens/core), full weights.
Matmuls run in bf16 (psum accumulation stays fp32); LN statistics and the
residual spine stay fp32.
"""
import sys

sys.path.insert(0, "/opt/trn_rl_repo")

import numpy as np
import ml_dtypes

import concourse.bass as bass
import concourse.mybir as mybir
import concourse.tile as tile
from concourse import bacc
from concourse.bass_utils import run_bass_kernel_spmd
from concourse.masks import make_identity

AF = mybir.ActivationFunctionType
ALU = mybir.AluOpType
F32 = mybir.dt.float32
F32R = mybir.dt.float32r
BF16 = mybir.dt.bfloat16

B, S, D, H = 2, 2048, 1024, 16
DH = D // H          # 64
DFF = 4 * D          # 4096
EPS = 1e-5
NC = 8               # cores
G = 4                # cores per group (per batch)
TS = S // G          # 512 tokens per strip
HC = H // G          # 4 heads per core
CC = HC * DH         # 256 head-columns per core
P = 128
KD = D // P          # 8 k-tiles over D
KF = DFF // P        # 32 k-tiles over DFF
NTT = S // P         # 16 token tiles (full sequence)
NT = TS // P         # 4 token tiles per strip
GROUPS = [[0, 1, 2, 3], [4, 5, 6, 7]]

_CACHE = {}


def build():
    nc = bacc.Bacc(None)

    io = {}
    io["x_d"] = nc.declare_dram_parameter("x", [S, D], F32, isOutput=False)
    io["xo_d"] = nc.declare_dram_parameter("xo", [TS, D], F32, isOutput=False)
    io["ln1g_d"] = nc.declare_dram_parameter("ln1_g", [D], F32, isOutput=False)
    io["ln1b_d"] = nc.declare_dram_parameter("ln1_b", [D], F32, isOutput=False)
    io["wq_d"] = nc.declare_dram_parameter("Wq", [D, CC], BF16, isOutput=False)
    io["wk_d"] = nc.declare_dram_parameter("Wk", [D, CC], BF16, isOutput=False)
    io["wv_d"] = nc.declare_dram_parameter("Wv", [D, CC], BF16, isOutput=False)
    io["bq_d"] = nc.declare_dram_parameter("bq", [CC], F32, isOutput=False)
    io["bk_d"] = nc.declare_dram_parameter("bk", [CC], F32, isOutput=False)
    io["bv_d"] = nc.declare_dram_parameter("bv", [CC], F32, isOutput=False)
    io["wo_d"] = nc.declare_dram_parameter("Wo", [CC, D], BF16, isOutput=False)
    io["bo_d"] = nc.declare_dram_parameter("bo", [D], F32, isOutput=False)
    io["ln2g_d"] = nc.declare_dram_parameter("ln2_g", [D], F32, isOutput=False)
    io["ln2b_d"] = nc.declare_dram_parameter("ln2_b", [D], F32, isOutput=False)
    io["w1_d"] = nc.declare_dram_parameter("W1", [D, DFF], BF16, isOutput=False)
    io["b1_d"] = nc.declare_dram_parameter("b1", [DFF], F32, isOutput=False)
    io["w2_d"] = nc.declare_dram_parameter("W2", [DFF, D], BF16, isOutput=False)
    io["b2_d"] = nc.declare_dram_parameter("b2", [D], F32R, isOutput=False)
    io["cmask_d"] = nc.declare_dram_parameter("cmask", [P, 4, 2 * TS], BF16,
                                              isOutput=False)
    io["y_d"] = nc.declare_dram_parameter("y", [TS, D], F32, isOutput=True)

    io["rs_in"] = nc.dram_tensor("rs_in", [S, D], BF16)
    io["rs_out"] = nc.dram_tensor("rs_out", [TS, D], BF16)

    with tile.TileContext(nc) as tc:
        _body(nc, tc, io)
    nc.compile()
    return nc


def _body(nc, tc, t):
    with tc.tile_pool(name="const", bufs=1) as cst:
        # ---- tiny constants + the LN1 params gate phase A: issue first ----
        ln1g = cst.tile([P, KD], F32)
        ln1b = cst.tile([P, KD], F32)
        nc.sync.dma_start(ln1g[:], t["ln1g_d"].rearrange("(k p) -> p k", p=P))
        nc.sync.dma_start(ln1b[:], t["ln1b_d"].rearrange("(k p) -> p k", p=P))

        identb = cst.tile([P, P], BF16)
        make_identity(nc, identb[:])
        epsc = cst.tile([P, 1], F32)
        nc.gpsimd.memset(epsc[:], EPS)
        onesrow_f = cst.tile([1, P], F32)
        nc.gpsimd.memset(onesrow_f[:], 1.0)
        ones128 = cst.tile([1, P], F32R)
        nc.vector.tensor_copy(ones128[:], onesrow_f[:])
        onescol4 = cst.tile([P, HC, 1], F32)
        nc.gpsimd.memset(onescol4[:], 1.0)

        # ---------------- helpers ----------------
        def layernorm(src_tiles, dst_tiles, sc, sq):
            # var = E[x^2] - mu^2; normalize is one fused (x - mu) * inv pass.
            # reduce/normalize alternate between DVE and Pool to halve the
            # DVE serial time; the Square/Sqrt chain stays on Act.
            n = len(src_tiles)
            for mt in range(n):
                ve = nc.vector if mt % 2 == 0 else nc.gpsimd
                xt = src_tiles[mt]
                mu = sc.tile([P, 1], F32, tag="mu", name="mu")
                nc.vector.tensor_reduce(out=mu[:], in_=xt[:], op=ALU.add,
                                        axis=mybir.AxisListType.X)
                mus = sc.tile([P, 1], F32, tag="mus", name="mus")
                nc.scalar.mul(mus[:], mu[:], 1.0 / D)
                sumsq = sc.tile([P, 1], F32, tag="sumsq", name="sumsq")
                nc.scalar.activation(sq[:], xt[:], AF.Square, accum_out=sumsq[:])
                mu2 = sc.tile([P, 1], F32, tag="mu2", name="mu2")
                nc.scalar.activation(mu2[:], mus[:], AF.Square)
                vpe = sc.tile([P, 1], F32, tag="vpe", name="vpe")
                ve.tensor_scalar(out=vpe[:], in0=sumsq[:],
                                 scalar1=1.0 / D, scalar2=mu2[:],
                                 op0=ALU.mult, op1=ALU.subtract)
                std = sc.tile([P, 1], F32, tag="std", name="std")
                nc.scalar.activation(std[:], vpe[:], AF.Sqrt, bias=epsc[:])
                inv = sc.tile([P, 1], F32, tag="inv", name="inv")
                nc.vector.reciprocal(inv[:], std[:])
                ve.tensor_scalar(out=dst_tiles[mt][:], in0=xt[:],
                                 scalar1=mus[:], scalar2=inv[:],
                                 op0=ALU.subtract, op1=ALU.mult)

        def transpose_strip(h_tiles, dst, g_sb, b_sb, tp, k, base_mt, nmt,
                            evict_act):
            # transpose nmt 128x128 blocks of column-tile k into one psum
            # bank, then evict once with the fused *g+b (DVE or Act).
            ps = tp.tile([P, NT * P], BF16, tag="tps", name="tps")
            for j in range(nmt):
                nc.tensor.transpose(ps[:, j * P:(j + 1) * P],
                                    h_tiles[base_mt + j][:, k * P:(k + 1) * P],
                                    identb[:])
            w = nmt * P
            if evict_act:
                nc.scalar.activation(dst[:, :w], ps[:, :w], AF.Identity,
                                     bias=b_sb[:, k:k + 1],
                                     scale=g_sb[:, k:k + 1])
            else:
                nc.vector.tensor_scalar(out=dst[:, :w], in0=ps[:, :w],
                                        scalar1=g_sb[:, k:k + 1],
                                        scalar2=b_sb[:, k:k + 1],
                                        op0=ALU.mult, op1=ALU.add)

        # wo + masks live in the const pool
        wot = cst.tile([P, 2, D], BF16, tag="wot", name="wot")
        wo = [wot[:, kk, :] for kk in range(2)]
        cmaskt = cst.tile([P, 4, 2 * TS], BF16, tag="cmask", name="cmask")

        # ============ persistent pools (LIFO lifetimes) ============
        xop_cm = tc.tile_pool(name="xop", bufs=1)
        xop = xop_cm.__enter__()
        xot = xop.tile([P, NT, D], F32, tag="xot", name="xot")
        xo = [xot[:, mt, :] for mt in range(NT)]

        w1p_cm = tc.tile_pool(name="w1p", bufs=1)
        w1p = w1p_cm.__enter__()
        w1t = w1p.tile([P, KD, DFF], BF16, tag="w1t", name="w1t")
        w1s = [w1t[:, k, :] for k in range(KD)]

        qkvp_cm = tc.tile_pool(name="qkvP", bufs=1)
        qkv = qkvp_cm.__enter__()
        qT = [qkv.tile([P, S], BF16, tag=f"qT{m}", name=f"qT{m}") for m in range(2)]
        kT = [qkv.tile([P, S], BF16, tag=f"kT{m}", name=f"kT{m}") for m in range(2)]
        vo = [qkv.tile([P, HC, DH + 1], BF16, tag=f"vo{tm}", name=f"vo{tm}")
              for tm in range(NTT)]
        aT = [qkv.tile([P, S], BF16, tag=f"aT{kk}", name=f"aT{kk}")
              for kk in range(2)]

        wqkvp_cm = tc.tile_pool(name="wqkv", bufs=1)
        wqkvp = wqkvp_cm.__enter__()
        wqt = wqkvp.tile([P, KD, CC], BF16, tag="wqt", name="wqt")
        wkt = wqkvp.tile([P, KD, CC], BF16, tag="wkt", name="wkt")
        wvt = wqkvp.tile([P, KD, CC], BF16, tag="wvt", name="wvt")
        wq = [wqt[:, k, :] for k in range(KD)]
        wk = [wkt[:, k, :] for k in range(KD)]
        wv = [wvt[:, k, :] for k in range(KD)]

        hTp_cm = tc.tile_pool(name="hTp", bufs=1)
        hTp = hTp_cm.__enter__()
        hT = [hTp.tile([P, S], BF16, tag=f"hT{k}", name=f"hT{k}")
              for k in range(KD)]

        # ============ phase A: x stream + LN1 + transpose ============
        with tc.tile_pool(name="xs", bufs=2) as xsp, \
             tc.tile_pool(name="hs", bufs=1) as hsp, \
             tc.tile_pool(name="sqA", bufs=1) as sqp, \
             tc.tile_pool(name="lnA", bufs=2) as sc, \
             tc.tile_pool(name="tpA", bufs=2, space="PSUM") as tpA:
            sqA = sqp.tile([P, D], F32, tag="sqA", name="sqA")
            for mtg in range(NTT // NT):
                hcur = []
                for j in range(NT):
                    mt = mtg * NT + j
                    xt = xsp.tile([P, D], F32, tag="x", name="x")
                    nc.sync.dma_start(xt[:], t["x_d"][mt * P:(mt + 1) * P, :])
                    ht = hsp.tile([P, D], BF16, tag=f"h{j}", name=f"h{j}")
                    layernorm([xt], [ht], sc, sqA)
                    hcur.append(ht)
                for k in range(KD):
                    transpose_strip(hcur,
                                    hT[k][:, mtg * NT * P:(mtg + 1) * NT * P],
                                    ln1g, ln1b, tpA, k, 0, NT,
                                    evict_act=(k % 2 == 1))

            # bulk prefetch: one rearranged DMA per tensor, SP-issued after
            # the x stream so they don't starve it (DMA idles during B/C)
            nc.sync.dma_start(xot[:], t["xo_d"].rearrange("(i p) d -> p i d", p=P))
            nc.sync.dma_start(wqt[:], t["wq_d"].rearrange("(k p) c -> p k c", p=P))
            nc.sync.dma_start(wkt[:], t["wk_d"].rearrange("(k p) c -> p k c", p=P))
            nc.sync.dma_start(wvt[:], t["wv_d"].rearrange("(k p) c -> p k c", p=P))
            nc.sync.dma_start(wot[:], t["wo_d"].rearrange("(k p) d -> p k d", p=P))
            nc.sync.dma_start(w1t[:], t["w1_d"].rearrange("(k p) f -> p k f", p=P))
            nc.sync.dma_start(cmaskt[:], t["cmask_d"][:])

        # remaining small constants (issued behind x on SP; needed later)
        ln2g = cst.tile([P, KD], F32)
        ln2b = cst.tile([P, KD], F32)
        nc.sync.dma_start(ln2g[:], t["ln2g_d"].rearrange("(k p) -> p k", p=P))
        nc.sync.dma_start(ln2b[:], t["ln2b_d"].rearrange("(k p) -> p k", p=P))
        bqp = cst.tile([P, 2], F32)
        bkp = cst.tile([P, 2], F32)
        nc.sync.dma_start(bqp[:], t["bq_d"].rearrange("(m p) -> p m", p=P))
        nc.sync.dma_start(bkp[:], t["bk_d"].rearrange("(m p) -> p m", p=P))
        bvrow = cst.tile([1, CC], F32)
        nc.sync.dma_start(bvrow[:], t["bv_d"][None, :])
        bvb = cst.tile([P, CC], F32)
        nc.gpsimd.partition_broadcast(bvb[:], bvrow[:])
        borow = cst.tile([1, D], F32)
        nc.sync.dma_start(borow[:], t["bo_d"][None, :])
        bob = cst.tile([P, D], F32)
        nc.gpsimd.partition_broadcast(bob[:], borow[:])
        b1p = cst.tile([P, KF], F32)
        nc.sync.dma_start(b1p[:], t["b1_d"].rearrange("(k p) -> p k", p=P))
        b2r = cst.tile([1, D], F32R)
        nc.sync.dma_start(b2r[:], t["b2_d"][None, :])


        # ============ phase B: QKV projections ============
        with tc.tile_pool(name="projPS", bufs=2, space="PSUM") as pps, \
             tc.tile_pool(name="vPS", bufs=2, space="PSUM") as vps:
            for (w_sb, b_sb, out_sb) in ((wq, bqp, qT), (wk, bkp, kT)):
                for m in range(2):
                    for st in range(G):
                        ps = pps.tile([P, TS], F32, tag="pp", name="pp")
                        for k in range(KD):
                            nc.tensor.matmul(
                                ps[:], w_sb[k][:, m * P:(m + 1) * P],
                                hT[k][:, st * TS:(st + 1) * TS],
                                start=(k == 0), stop=(k == KD - 1))
                        if st % 2 == 0:
                            nc.vector.tensor_scalar(
                                out=out_sb[m][:, st * TS:(st + 1) * TS],
                                in0=ps[:], scalar1=b_sb[:, m:m + 1],
                                scalar2=None, op0=ALU.add)
                        else:
                            nc.scalar.activation(
                                out_sb[m][:, st * TS:(st + 1) * TS], ps[:],
                                AF.Identity, bias=b_sb[:, m:m + 1])
            for tm in range(NTT):
                ps = vps.tile([P, CC], F32, tag="vp", name="vp")
                for k in range(KD):
                    nc.tensor.matmul(
                        ps[:], hT[k][:, tm * P:(tm + 1) * P], wv[k][:],
                        start=(k == 0), stop=(k == KD - 1))
                nc.vector.tensor_tensor(
                    out=vo[tm][:, :, 0:DH],
                    in0=ps[:].rearrange("p (h e) -> p h e", h=HC),
                    in1=bvb[:].rearrange("p (h e) -> p h e", h=HC),
                    op=ALU.add)
                nc.vector.tensor_copy(vo[tm][:, :, DH:DH + 1], onescol4[:])

        hTp_cm.__exit__(None, None, None)
        wqkvp_cm.__exit__(None, None, None)

        # ===== phases C+D: attention (qc-outer) + fused out_proj =====
        # per query strip: both head pairs' attention, then that strip's
        # out_proj partial immediately (fills PE while Act runs exp).
        # kt loop is software-pipelined: scores(kt+1) is traced before AV(kt)
        # so PE isn't idle while Act computes exp(kt).
        with (
            tc.tile_pool(name="scPS", bufs=2, space="PSUM") as scp,
            tc.tile_pool(name="avPS", bufs=1, space="PSUM") as avp,
            tc.tile_pool(name="opPS", bufs=1, space="PSUM") as opp,
            tc.tile_pool(name="attnSB", bufs=3) as asb,
            tc.tile_pool(name="opSB", bufs=3) as osb,
        ):
            for qc in range(G):
                kt_max = 4 * qc + 3
                for hp in range(HC // 2):      # head pairs at PE rows 0/64
                    avs = [avp.tile([DH + 1, TS], F32, tag=f"av{j}",
                                    name=f"av{j}") for j in range(2)]

                    def scores(kt):
                        w0 = P * max(0, kt - 4 * qc)
                        sc_ps = scp.tile([P, 2, TS], F32, tag="scp", name="scp")
                        for j in range(2):
                            o = j * DH
                            nc.tensor.matmul(
                                sc_ps[:, j, w0:],
                                kT[hp][o:o + DH, kt * P:(kt + 1) * P],
                                qT[hp][o:o + DH, qc * TS + w0:(qc + 1) * TS],
                                start=True, stop=True)
                        return sc_ps

                    def expmask(kt, sc_ps):
                        e_r = asb.tile([P, 2, TS], BF16, tag="erp", name="erp")
                        if kt < 4 * qc:
                            v0 = 0      # valid columns start
                            nc.scalar.activation(
                                e_r[:].rearrange("p a b -> p (a b)"),
                                sc_ps[:].rearrange("p a b -> p (a b)"),
                                AF.Exp, scale=0.125)
                        else:
                            # diag block d: cols < 128*d are fully masked --
                            # never compute/read them
                            d = kt - 4 * qc
                            v0 = P * d
                            e_f = asb.tile([P, 2, TS], BF16, tag="efp",
                                           name="efp")
                            nc.scalar.activation(
                                e_f[:, :, v0:], sc_ps[:, :, v0:],
                                AF.Exp, scale=0.125)
                            mdv = cmaskt[:, d, :].rearrange(
                                "p (a b) -> p a b", a=2)
                            nc.vector.tensor_tensor(
                                out=e_r[:, :, v0:], in0=e_f[:, :, v0:],
                                in1=mdv[:, :, v0:], op=ALU.mult)
                        return e_r, v0

                    sc_prev = scores(0)
                    for kt in range(kt_max + 1):
                        e_r, v0 = expmask(kt, sc_prev)
                        if kt < kt_max:
                            sc_prev = scores(kt + 1)
                        for j in range(2):
                            nc.tensor.matmul(avs[j][:, v0:],
                                             vo[kt][:, 2 * hp + j, :],
                                             e_r[:, j, v0:],
                                             start=(kt == 0),
                                             stop=(kt == kt_max))
                    for j in range(2):
                        rec = asb.tile([1, TS], F32, tag=f"rec{j}",
                                       name=f"rec{j}")
                        nc.vector.reciprocal(rec[:], avs[j][DH:DH + 1, :])
                        rb = asb.tile([DH, TS], F32, tag=f"rb{j}",
                                      name=f"rb{j}")
                        nc.gpsimd.partition_broadcast(rb[:], rec[:])
                        nc.vector.tensor_tensor(
                            out=aT[hp][j * DH:(j + 1) * DH,
                                       qc * TS:(qc + 1) * TS],
                            in0=avs[j][0:DH, :], in1=rb[:], op=ALU.mult)

                # out_proj partial for this strip's tokens
                for mt in range(qc * NT, (qc + 1) * NT):
                    ps = opp.tile([P, 2, TS], F32, tag="op", name="op")
                    for n in range(2):
                        for kk in range(2):
                            nc.tensor.matmul(
                                ps[:, n, :], aT[kk][:, mt * P:(mt + 1) * P],
                                wo[kk][:, n * TS:(n + 1) * TS],
                                start=(kk == 0), stop=(kk == 1))
                    ot = osb.tile([P, D], BF16, tag="ot", name="ot")
                    if mt % 2 == 0:
                        nc.vector.tensor_copy(
                            ot[:], ps[:].rearrange("p a b -> p (a b)"))
                    else:
                        nc.scalar.activation(
                            ot[:], ps[:].rearrange("p a b -> p (a b)"), AF.Copy)
                    nc.sync.dma_start(t["rs_in"][mt * P:(mt + 1) * P, :], ot[:])

        qkvp_cm.__exit__(None, None, None)

        # ============ ReduceScatter: sum head-partials, keep own strip ======
        nc.gpsimd.collective_compute(
            "ReduceScatter", ALU.add, ins=[t["rs_in"][:]],
            outs=[t["rs_out"][:]], replica_groups=GROUPS,
        )

        # ============ phase E: residual + LN2 + FFN (token-sharded) ========
        # W2 stream pool opens first (LIFO: closes last); prefetch half of W2
        # on the Act DGE queue so it transfers during the ReduceScatter.
        w2p_cm = tc.tile_pool(name="w2st", bufs=16)
        w2p = w2p_cm.__enter__()
        w2ts = {}
        for k2 in range(KF // 2):
            w2t = w2p.tile([P, D], BF16, tag="w2", name="w2")
            nc.scalar.dma_start(w2t[:], t["w2_d"][k2 * P:(k2 + 1) * P, :])
            w2ts[k2] = w2t

        gTp_cm = tc.tile_pool(name="gTp", bufs=1)
        gtp = gTp_cm.__enter__()
        gT = [gtp.tile([P, TS], BF16, tag=f"gT{mf}", name=f"gT{mf}")
              for mf in range(KF)]
        h2Tp_cm = tc.tile_pool(name="h2Tp", bufs=1)
        h2tp = h2Tp_cm.__enter__()
        h2T = [h2tp.tile([P, TS], BF16, tag=f"h2T{k}", name=f"h2T{k}")
               for k in range(KD)]

        with tc.tile_pool(name="rsb", bufs=1) as rsb, \
             tc.tile_pool(name="lnD", bufs=2) as sc2, \
             tc.tile_pool(name="h2P", bufs=1) as h2sp, \
             tc.tile_pool(name="tpD", bufs=2, space="PSUM") as tpD:
            rstt = rsb.tile([P, NT, D], BF16, tag="rst", name="rst")
            rst = [rstt[:, mt, :] for mt in range(NT)]
            nc.sync.dma_start(rstt[:],
                              t["rs_out"].rearrange("(i p) d -> p i d", p=P))
            for mt in range(NT):
                ve = nc.vector if mt % 2 == 0 else nc.gpsimd
                ve.tensor_tensor(out=xo[mt][:], in0=xo[mt][:], in1=rst[mt][:],
                                 op=ALU.add)
                ve.tensor_tensor(out=xo[mt][:], in0=xo[mt][:], in1=bob[:],
                                 op=ALU.add)
            sqD = h2sp.tile([P, D], F32, tag="sqD", name="sqD")
            h2 = [h2sp.tile([P, D], BF16, tag=f"h2{mt}", name=f"h2{mt}")
                  for mt in range(NT)]
            layernorm(xo, h2, sc2, sqD)
            for k in range(KD):
                transpose_strip(h2, h2T[k][:], ln2g, ln2b, tpD, k, 0, NT,
                                evict_act=(k % 2 == 1))

        # ---- fc1 + gelu ----
        with tc.tile_pool(name="gPS", bufs=4, space="PSUM") as gps:
            for mf in range(KF):
                ps = gps.tile([P, TS], F32, tag="g", name="g")
                for k in range(KD):
                    nc.tensor.matmul(
                        ps[:], w1s[k][:, mf * P:(mf + 1) * P], h2T[k][:],
                        start=(k == 0), stop=(k == KD - 1))
                nc.scalar.activation(gT[mf][:], ps[:], AF.Gelu,
                                     bias=b1p[:, mf:mf + 1])
        h2Tp_cm.__exit__(None, None, None)

        # ---- fc2 + residual ----
        with tc.tile_pool(name="fPS", bufs=1, space="PSUM") as fps, \
             tc.tile_pool(name="ySB", bufs=2) as ysb:
            f_ps = [fps.tile([P, 2, TS], F32, tag=f"f{mt}", name=f"f{mt}")
                    for mt in range(NT)]
            for mt in range(NT):
                for n in range(2):
                    nc.tensor.matmul(
                        f_ps[mt][:, n, :], ones128[:],
                        b2r[:, n * TS:(n + 1) * TS], start=True, stop=False)
            for k2 in range(KF):
                if k2 in w2ts:
                    w2t = w2ts[k2]
                else:
                    w2t = w2p.tile([P, D], BF16, tag="w2", name="w2")
                    nc.scalar.dma_start(w2t[:],
                                        t["w2_d"][k2 * P:(k2 + 1) * P, :])
                for mt in range(NT):
                    for n in range(2):
                        nc.tensor.matmul(
                            f_ps[mt][:, n, :],
                            gT[k2][:, mt * P:(mt + 1) * P],
                            w2t[:, n * TS:(n + 1) * TS],
                            start=False, stop=(k2 == KF - 1))
            for mt in range(NT):
                yt = ysb.tile([P, D], F32, tag="y", name="y")
                nc.vector.tensor_tensor(
                    out=yt[:], in0=f_ps[mt][:].rearrange("p a b -> p (a b)"),
                    in1=xo[mt][:], op=ALU.add)
                nc.sync.dma_start(t["y_d"][mt * P:(mt + 1) * P, :], yt[:])
        gTp_cm.__exit__(None, None, None)
        w2p_cm.__exit__(None, None, None)
        w1p_cm.__exit__(None, None, None)
        xop_cm.__exit__(None, None, None)


def _in_maps(inputs):
    f32 = np.float32
    bf16 = ml_dtypes.bfloat16

    def as_bf16(a):
        return np.ascontiguousarray(np.asarray(a, f32).astype(bf16))

    x = np.asarray(inputs["x"], f32)
    # causal masks for the 4 diagonal sub-blocks: keep where col >= p + 128*d
    cmask = np.zeros((P, 4, 2 * TS), np.float32)
    cols = np.arange(TS)[None, :]
    rows = np.arange(P)[:, None]
    for d in range(4):
        m = (cols >= rows + 128 * d).astype(np.float32)
        cmask[:, d, 0:TS] = m
        cmask[:, d, TS:2 * TS] = m
    cmask = cmask.astype(bf16)
    maps = []
    for c in range(NC):
        b, r = c // G, c % G
        c0 = r * CC
        m = {
            "x": np.ascontiguousarray(x[b]),
            "xo": np.ascontiguousarray(x[b, r * TS:(r + 1) * TS, :]),
            "ln1_g": np.ascontiguousarray(inputs["ln1_g"], f32),
            "ln1_b": np.ascontiguousarray(inputs["ln1_b"], f32),
            "Wq": as_bf16(np.asarray(inputs["Wq"], f32)[:, c0:c0 + CC]),
            "Wk": as_bf16(np.asarray(inputs["Wk"], f32)[:, c0:c0 + CC]),
            "Wv": as_bf16(np.asarray(inputs["Wv"], f32)[:, c0:c0 + CC]),
            "bq": np.ascontiguousarray(np.asarray(inputs["bq"], f32)[c0:c0 + CC]),
            "bk": np.ascontiguousarray(np.asarray(inputs["bk"], f32)[c0:c0 + CC]),
            "bv": np.ascontiguousarray(np.asarray(inputs["bv"], f32)[c0:c0 + CC]),
            "Wo": as_bf16(np.asarray(inputs["Wo"], f32)[c0:c0 + CC, :]),
            "bo": np.ascontiguousarray(inputs["bo"], f32),
            "ln2_g": np.ascontiguousarray(inputs["ln2_g"], f32),
            "ln2_b": np.ascontiguousarray(inputs["ln2_b"], f32),
            "W1": as_bf16(inputs["W1"]),
            "b1": np.ascontiguousarray(inputs["b1"], f32),
            "W2": as_bf16(inputs["W2"]),
            "b2": np.ascontiguousarray(inputs["b2"], f32),
            "cmask": cmask,
        }
        maps.append(m)
    return maps


def _run(inputs, trace=False):
    if "nc" not in _CACHE:
        _CACHE["nc"] = build()
    nc = _CACHE["nc"]
    maps = _in_maps(inputs)
    res = run_bass_kernel_spmd(nc, maps, list(range(NC)), trace=trace)
    out = np.empty((B, S, D), np.float32)
    for c in range(NC):
        b, r = c // G, c % G
        out[b, r * TS:(r + 1) * TS, :] = res.results[c]["y"]
    return out, res


def kernel(**inputs):
    out, _ = _run(inputs, trace=False)
    return out


if __name__ == "__main__":
    build()
    print("build OK")


# revision 14
# speedup vs baseline: 1.8704x; 1.0187x over previous
"""Trainium2 Bass kernel for a dense transformer block (B=2,S=2048,D=1024,H=16,DFF=4096).

Sharding across 8 NeuronCores:
  core c: batch b=c//4, group rank r=c%4, replica groups [[0,1,2,3],[4,5,6,7]].
  - Every core loads the FULL x of its batch; LN1 + transpose are replicated
    (no collective needed for the attention input).
  - Attention: head-parallel (4 heads/core, full causal sequence), output
    kept on-chip.
  - out_proj: each core computes the partial sum over its own heads for ALL
    2048 tokens; a single ReduceScatter(add) in bf16 then hands each core its
    own summed 512-token strip (output bytes 1MB vs the 8MB an AllGather of
    the attention output would move).
  - residual, LN2, FFN: token-sharded (512 tokens/core), full weights.
Matmuls run in bf16 (psum accumulation stays fp32); LN statistics and the
residual spine stay fp32.
"""
import sys

sys.path.insert(0, "/opt/trn_rl_repo")

import numpy as np
import ml_dtypes

import concourse.bass as bass
import concourse.mybir as mybir
import concourse.tile as tile
from concourse import bacc
from concourse.bass_utils import run_bass_kernel_spmd
from concourse.masks import make_identity

AF = mybir.ActivationFunctionType
ALU = mybir.AluOpType
F32 = mybir.dt.float32
F32R = mybir.dt.float32r
BF16 = mybir.dt.bfloat16

B, S, D, H = 2, 2048, 1024, 16
DH = D // H          # 64
DFF = 4 * D          # 4096
EPS = 1e-5
NC = 8               # cores
G = 4                # cores per group (per batch)
TS = S // G          # 512 tokens per strip
HC = H // G          # 4 heads per core
CC = HC * DH         # 256 head-columns per core
P = 128
KD = D // P          # 8 k-tiles over D
KF = DFF // P        # 32 k-tiles over DFF
NTT = S // P         # 16 token tiles (full sequence)
NT = TS // P         # 4 token tiles per strip
GROUPS = [[0, 1, 2, 3], [4, 5, 6, 7]]

_CACHE = {}


def build():
    nc = bacc.Bacc(None)

    io = {}
    io["x_d"] = nc.declare_dram_parameter("x", [S, D], F32, isOutput=False)
    io["xo_d"] = nc.declare_dram_parameter("xo", [TS, D], F32, isOutput=False)
    io["ln1g_d"] = nc.declare_dram_parameter("ln1_g", [D], F32, isOutput=False)
    io["ln1b_d"] = nc.declare_dram_parameter("ln1_b", [D], F32, isOutput=False)
    io["wq_d"] = nc.declare_dram_parameter("Wq", [D, CC], BF16, isOutput=False)
    io["wk_d"] = nc.declare_dram_parameter("Wk", [D, CC], BF16, isOutput=False)
    io["wv_d"] = nc.declare_dram_parameter("Wv", [D, CC], BF16, isOutput=False)
    io["bq_d"] = nc.declare_dram_parameter("bq", [CC], F32, isOutput=False)
    io["bk_d"] = nc.declare_dram_parameter("bk", [CC], F32, isOutput=False)
    io["bv_d"] = nc.declare_dram_parameter("bv", [CC], F32, isOutput=False)
    io["wo_d"] = nc.declare_dram_parameter("Wo", [CC, D], BF16, isOutput=False)
    io["bo_d"] = nc.declare_dram_parameter("bo", [D], F32, isOutput=False)
    io["ln2g_d"] = nc.declare_dram_parameter("ln2_g", [D], F32, isOutput=False)
    io["ln2b_d"] = nc.declare_dram_parameter("ln2_b", [D], F32, isOutput=False)
    io["w1_d"] = nc.declare_dram_parameter("W1", [D, DFF], BF16, isOutput=False)
    io["b1_d"] = nc.declare_dram_parameter("b1", [DFF], F32, isOutput=False)
    io["w2_d"] = nc.declare_dram_parameter("W2", [DFF, D], BF16, isOutput=False)
    io["b2_d"] = nc.declare_dram_parameter("b2", [D], F32R, isOutput=False)
    io["cmask_d"] = nc.declare_dram_parameter("cmask", [P, 4, 2 * TS], BF16,
                                              isOutput=False)
    io["y_d"] = nc.declare_dram_parameter("y", [TS, D], F32, isOutput=True)

    io["rs_in"] = nc.dram_tensor("rs_in", [S, D], BF16)
    io["rs_out"] = nc.dram_tensor("rs_out", [TS, D], BF16)

    with tile.TileContext(nc) as tc:
        _body(nc, tc, io)
    nc.compile()
    return nc


def _body(nc, tc, t):
    with tc.tile_pool(name="const", bufs=1) as cst:
        # ---- tiny constants + the LN1 params gate phase A: issue first ----
        ln1g = cst.tile([P, KD], F32)
        ln1b = cst.tile([P, KD], F32)
        nc.sync.dma_start(ln1g[:], t["ln1g_d"].rearrange("(k p) -> p k", p=P))
        nc.sync.dma_start(ln1b[:], t["ln1b_d"].rearrange("(k p) -> p k", p=P))

        identb = cst.tile([P, P], BF16)
        make_identity(nc, identb[:])
        epsc = cst.tile([P, 1], F32)
        nc.gpsimd.memset(epsc[:], EPS)
        onesrow_f = cst.tile([1, P], F32)
        nc.gpsimd.memset(onesrow_f[:], 1.0)
        ones128 = cst.tile([1, P], F32R)
        nc.vector.tensor_copy(ones128[:], onesrow_f[:])
        onescol4 = cst.tile([P, HC, 1], F32)
        nc.gpsimd.memset(onescol4[:], 1.0)

        # ---------------- helpers ----------------
        def layernorm(src_tiles, dst_tiles, sc, sq):
            # var = E[x^2] - mu^2; normalize is one fused (x - mu) * inv pass.
            # reduce/normalize alternate between DVE and Pool to halve the
            # DVE serial time; the Square/Sqrt chain stays on Act.
            n = len(src_tiles)
            for mt in range(n):
                ve = nc.vector if mt % 2 == 0 else nc.gpsimd
                xt = src_tiles[mt]
                mu = sc.tile([P, 1], F32, tag="mu", name="mu")
                nc.vector.tensor_reduce(out=mu[:], in_=xt[:], op=ALU.add,
                                        axis=mybir.AxisListType.X)
                mus = sc.tile([P, 1], F32, tag="mus", name="mus")
                nc.scalar.mul(mus[:], mu[:], 1.0 / D)
                sumsq = sc.tile([P, 1], F32, tag="sumsq", name="sumsq")
                nc.scalar.activation(sq[:], xt[:], AF.Square, accum_out=sumsq[:])
                mu2 = sc.tile([P, 1], F32, tag="mu2", name="mu2")
                nc.scalar.activation(mu2[:], mus[:], AF.Square)
                vpe = sc.tile([P, 1], F32, tag="vpe", name="vpe")
                ve.tensor_scalar(out=vpe[:], in0=sumsq[:],
                                 scalar1=1.0 / D, scalar2=mu2[:],
                                 op0=ALU.mult, op1=ALU.subtract)
                std = sc.tile([P, 1], F32, tag="std", name="std")
                nc.scalar.activation(std[:], vpe[:], AF.Sqrt, bias=epsc[:])
                inv = sc.tile([P, 1], F32, tag="inv", name="inv")
                nc.vector.reciprocal(inv[:], std[:])
                ve.tensor_scalar(out=dst_tiles[mt][:], in0=xt[:],
                                 scalar1=mus[:], scalar2=inv[:],
                                 op0=ALU.subtract, op1=ALU.mult)

        def transpose_strip(h_tiles, dst, g_sb, b_sb, tp, k, base_mt, nmt,
                            evict_act):
            # transpose nmt 128x128 blocks of column-tile k into one psum
            # bank, then evict once with the fused *g+b (DVE or Act).
            ps = tp.tile([P, NT * P], BF16, tag="tps", name="tps")
            for j in range(nmt):
                nc.tensor.transpose(ps[:, j * P:(j + 1) * P],
                                    h_tiles[base_mt + j][:, k * P:(k + 1) * P],
                                    identb[:])
            w = nmt * P
            if evict_act:
                nc.scalar.activation(dst[:, :w], ps[:, :w], AF.Identity,
                                     bias=b_sb[:, k:k + 1],
                                     scale=g_sb[:, k:k + 1])
            else:
                nc.vector.tensor_scalar(out=dst[:, :w], in0=ps[:, :w],
                                        scalar1=g_sb[:, k:k + 1],
                                        scalar2=b_sb[:, k:k + 1],
                                        op0=ALU.mult, op1=ALU.add)

        # wo + masks live in the const pool
        wot = cst.tile([P, 2, D], BF16, tag="wot", name="wot")
        wo = [wot[:, kk, :] for kk in range(2)]
        cmaskt = cst.tile([P, 4, 2 * TS], BF16, tag="cmask", name="cmask")

        # ============ persistent pools (LIFO lifetimes) ============
        xop_cm = tc.tile_pool(name="xop", bufs=1)
        xop = xop_cm.__enter__()
        xot = xop.tile([P, NT, D], F32, tag="xot", name="xot")
        xo = [xot[:, mt, :] for mt in range(NT)]

        w1p_cm = tc.tile_pool(name="w1p", bufs=1)
        w1p = w1p_cm.__enter__()
        w1t = w1p.tile([P, KD, DFF], BF16, tag="w1t", name="w1t")
        w1s = [w1t[:, k, :] for k in range(KD)]

        qkvp_cm = tc.tile_pool(name="qkvP", bufs=1)
        qkv = qkvp_cm.__enter__()
        qT = [qkv.tile([P, S], BF16, tag=f"qT{m}", name=f"qT{m}") for m in range(2)]
        kT = [qkv.tile([P, S], BF16, tag=f"kT{m}", name=f"kT{m}") for m in range(2)]
        vo = [qkv.tile([P, HC, DH + 1], BF16, tag=f"vo{tm}", name=f"vo{tm}")
              for tm in range(NTT)]
        aT = [qkv.tile([P, S], BF16, tag=f"aT{kk}", name=f"aT{kk}")
              for kk in range(2)]

        wqkvp_cm = tc.tile_pool(name="wqkv", bufs=1)
        wqkvp = wqkvp_cm.__enter__()
        wqt = wqkvp.tile([P, KD, CC], BF16, tag="wqt", name="wqt")
        wkt = wqkvp.tile([P, KD, CC], BF16, tag="wkt", name="wkt")
        wvt = wqkvp.tile([P, KD, CC], BF16, tag="wvt", name="wvt")
        wq = [wqt[:, k, :] for k in range(KD)]
        wk = [wkt[:, k, :] for k in range(KD)]
        wv = [wvt[:, k, :] for k in range(KD)]

        hTp_cm = tc.tile_pool(name="hTp", bufs=1)
        hTp = hTp_cm.__enter__()
        hT = [hTp.tile([P, S], BF16, tag=f"hT{k}", name=f"hT{k}")
              for k in range(KD)]

        # ============ phase A: x stream + LN1 + transpose ============
        with tc.tile_pool(name="xs", bufs=2) as xsp, \
             tc.tile_pool(name="hs", bufs=1) as hsp, \
             tc.tile_pool(name="sqA", bufs=1) as sqp, \
             tc.tile_pool(name="lnA", bufs=2) as sc, \
             tc.tile_pool(name="tpA", bufs=2, space="PSUM") as tpA:
            sqA = sqp.tile([P, D], F32, tag="sqA", name="sqA")
            for mtg in range(NTT // NT):
                hcur = []
                for j in range(NT):
                    mt = mtg * NT + j
                    xt = xsp.tile([P, D], F32, tag="x", name="x")
                    nc.sync.dma_start(xt[:], t["x_d"][mt * P:(mt + 1) * P, :])
                    ht = hsp.tile([P, D], BF16, tag=f"h{j}", name=f"h{j}")
                    layernorm([xt], [ht], sc, sqA)
                    hcur.append(ht)
                for k in range(KD):
                    transpose_strip(hcur,
                                    hT[k][:, mtg * NT * P:(mtg + 1) * NT * P],
                                    ln1g, ln1b, tpA, k, 0, NT,
                                    evict_act=(k % 2 == 1))

            # bulk prefetch: one rearranged DMA per tensor, SP-issued after
            # the x stream so they don't starve it (DMA idles during B/C)
            # chunked so they never block the paced x stream for long (the
            # DMA device serves by readiness, not issue order)
            for i in range(NT):
                nc.sync.dma_start(xot[:, i, :], t["xo_d"][i * P:(i + 1) * P, :])
            nc.sync.dma_start(wqt[:], t["wq_d"].rearrange("(k p) c -> p k c", p=P))
            nc.sync.dma_start(wkt[:], t["wk_d"].rearrange("(k p) c -> p k c", p=P))
            nc.sync.dma_start(wvt[:], t["wv_d"].rearrange("(k p) c -> p k c", p=P))
            nc.sync.dma_start(wot[:], t["wo_d"].rearrange("(k p) d -> p k d", p=P))
            for k in range(KD):
                nc.sync.dma_start(w1t[:, k, :], t["w1_d"][k * P:(k + 1) * P, :])
            nc.sync.dma_start(cmaskt[:], t["cmask_d"][:])

        # remaining small constants (issued behind x on SP; needed later)
        ln2g = cst.tile([P, KD], F32)
        ln2b = cst.tile([P, KD], F32)
        nc.sync.dma_start(ln2g[:], t["ln2g_d"].rearrange("(k p) -> p k", p=P))
        nc.sync.dma_start(ln2b[:], t["ln2b_d"].rearrange("(k p) -> p k", p=P))
        bqp = cst.tile([P, 2], F32)
        bkp = cst.tile([P, 2], F32)
        nc.sync.dma_start(bqp[:], t["bq_d"].rearrange("(m p) -> p m", p=P))
        nc.sync.dma_start(bkp[:], t["bk_d"].rearrange("(m p) -> p m", p=P))
        bvrow = cst.tile([1, CC], F32)
        nc.sync.dma_start(bvrow[:], t["bv_d"][None, :])
        bvb = cst.tile([P, CC], F32)
        nc.gpsimd.partition_broadcast(bvb[:], bvrow[:])
        borow = cst.tile([1, D], F32)
        nc.sync.dma_start(borow[:], t["bo_d"][None, :])
        bob = cst.tile([P, D], F32)
        nc.gpsimd.partition_broadcast(bob[:], borow[:])
        b1p = cst.tile([P, KF], F32)
        nc.sync.dma_start(b1p[:], t["b1_d"].rearrange("(k p) -> p k", p=P))
        b2r = cst.tile([1, D], F32R)
        nc.sync.dma_start(b2r[:], t["b2_d"][None, :])


        # ============ phase B: QKV projections ============
        with tc.tile_pool(name="projPS", bufs=2, space="PSUM") as pps, \
             tc.tile_pool(name="vPS", bufs=2, space="PSUM") as vps:
            for (w_sb, b_sb, out_sb) in ((wq, bqp, qT), (wk, bkp, kT)):
                for m in range(2):
                    for st in range(G):
                        ps = pps.tile([P, TS], F32, tag="pp", name="pp")
                        for k in range(KD):
                            nc.tensor.matmul(
                                ps[:], w_sb[k][:, m * P:(m + 1) * P],
                                hT[k][:, st * TS:(st + 1) * TS],
                                start=(k == 0), stop=(k == KD - 1))
                        if st % 2 == 0:
                            nc.vector.tensor_scalar(
                                out=out_sb[m][:, st * TS:(st + 1) * TS],
                                in0=ps[:], scalar1=b_sb[:, m:m + 1],
                                scalar2=None, op0=ALU.add)
                        else:
                            nc.scalar.activation(
                                out_sb[m][:, st * TS:(st + 1) * TS], ps[:],
                                AF.Identity, bias=b_sb[:, m:m + 1])
            for tm in range(NTT):
                ps = vps.tile([P, CC], F32, tag="vp", name="vp")
                for k in range(KD):
                    nc.tensor.matmul(
                        ps[:], hT[k][:, tm * P:(tm + 1) * P], wv[k][:],
                        start=(k == 0), stop=(k == KD - 1))
                nc.vector.tensor_tensor(
                    out=vo[tm][:, :, 0:DH],
                    in0=ps[:].rearrange("p (h e) -> p h e", h=HC),
                    in1=bvb[:].rearrange("p (h e) -> p h e", h=HC),
                    op=ALU.add)
                nc.vector.tensor_copy(vo[tm][:, :, DH:DH + 1], onescol4[:])

        hTp_cm.__exit__(None, None, None)
        wqkvp_cm.__exit__(None, None, None)

        # ===== phases C+D: attention (qc-outer) + fused out_proj =====
        # per query strip: both head pairs' attention, then that strip's
        # out_proj partial immediately (fills PE while Act runs exp).
        # kt loop is software-pipelined: scores(kt+1) is traced before AV(kt)
        # so PE isn't idle while Act computes exp(kt).
        with (
            tc.tile_pool(name="scPS", bufs=2, space="PSUM") as scp,
            tc.tile_pool(name="avPS", bufs=1, space="PSUM") as avp,
            tc.tile_pool(name="opPS", bufs=1, space="PSUM") as opp,
            tc.tile_pool(name="attnSB", bufs=3) as asb,
            tc.tile_pool(name="opSB", bufs=3) as osb,
        ):
            for qc in range(G):
                kt_max = 4 * qc + 3
                for hp in range(HC // 2):      # head pairs at PE rows 0/64
                    avs = [avp.tile([DH + 1, TS], F32, tag=f"av{j}",
                                    name=f"av{j}") for j in range(2)]

                    def scores(kt):
                        w0 = P * max(0, kt - 4 * qc)
                        sc_ps = scp.tile([P, 2, TS], F32, tag="scp", name="scp")
                        for j in range(2):
                            o = j * DH
                            nc.tensor.matmul(
                                sc_ps[:, j, w0:],
                                kT[hp][o:o + DH, kt * P:(kt + 1) * P],
                                qT[hp][o:o + DH, qc * TS + w0:(qc + 1) * TS],
                                start=True, stop=True)
                        return sc_ps

                    def expmask(kt, sc_ps):
                        e_r = asb.tile([P, 2, TS], BF16, tag="erp", name="erp")
                        if kt < 4 * qc:
                            v0 = 0      # valid columns start
                            nc.scalar.activation(
                                e_r[:].rearrange("p a b -> p (a b)"),
                                sc_ps[:].rearrange("p a b -> p (a b)"),
                                AF.Exp, scale=0.125)
                        else:
                            # diag block d: cols < 128*d are fully masked --
                            # never compute/read them
                            d = kt - 4 * qc
                            v0 = P * d
                            e_f = asb.tile([P, 2, TS], BF16, tag="efp",
                                           name="efp")
                            nc.scalar.activation(
                                e_f[:, :, v0:], sc_ps[:, :, v0:],
                                AF.Exp, scale=0.125)
                            mdv = cmaskt[:, d, :].rearrange(
                                "p (a b) -> p a b", a=2)
                            nc.vector.tensor_tensor(
                                out=e_r[:, :, v0:], in0=e_f[:, :, v0:],
                                in1=mdv[:, :, v0:], op=ALU.mult)
                        return e_r, v0

                    sc_prev = scores(0)
                    for kt in range(kt_max + 1):
                        e_r, v0 = expmask(kt, sc_prev)
                        if kt < kt_max:
                            sc_prev = scores(kt + 1)
                        for j in range(2):
                            nc.tensor.matmul(avs[j][:, v0:],
                                             vo[kt][:, 2 * hp + j, :],
                                             e_r[:, j, v0:],
                                             start=(kt == 0),
                                             stop=(kt == kt_max))
                    for j in range(2):
                        rec = asb.tile([1, TS], F32, tag=f"rec{j}",
                                       name=f"rec{j}")
                        nc.vector.reciprocal(rec[:], avs[j][DH:DH + 1, :])
                        rb = asb.tile([DH, TS], F32, tag=f"rb{j}",
                                      name=f"rb{j}")
                        nc.gpsimd.partition_broadcast(rb[:], rec[:])
                        nc.vector.tensor_tensor(
                            out=aT[hp][j * DH:(j + 1) * DH,
                                       qc * TS:(qc + 1) * TS],
                            in0=avs[j][0:DH, :], in1=rb[:], op=ALU.mult)

                # out_proj partial for this strip's tokens
                for mt in range(qc * NT, (qc + 1) * NT):
                    ps = opp.tile([P, 2, TS], F32, tag="op", name="op")
                    for n in range(2):
                        for kk in range(2):
                            nc.tensor.matmul(
                                ps[:, n, :], aT[kk][:, mt * P:(mt + 1) * P],
                                wo[kk][:, n * TS:(n + 1) * TS],
                                start=(kk == 0), stop=(kk == 1))
                    ot = osb.tile([P, D], BF16, tag="ot", name="ot")
                    if mt % 2 == 0:
                        nc.vector.tensor_copy(
                            ot[:], ps[:].rearrange("p a b -> p (a b)"))
                    else:
                        nc.scalar.activation(
                            ot[:], ps[:].rearrange("p a b -> p (a b)"), AF.Copy)
                    nc.sync.dma_start(t["rs_in"][mt * P:(mt + 1) * P, :], ot[:])

        qkvp_cm.__exit__(None, None, None)

        # ============ ReduceScatter: sum head-partials, keep own strip ======
        nc.gpsimd.collective_compute(
            "ReduceScatter", ALU.add, ins=[t["rs_in"][:]],
            outs=[t["rs_out"][:]], replica_groups=GROUPS,
        )

        # ============ phase E: residual + LN2 + FFN (token-sharded) ========
        # W2 stream pool opens first (LIFO: closes last); prefetch half of W2
        # on the Act DGE queue so it transfers during the ReduceScatter.
        w2p_cm = tc.tile_pool(name="w2st", bufs=16)
        w2p = w2p_cm.__enter__()
        w2ts = {}
        for k2 in range(KF // 2):
            w2t = w2p.tile([P, D], BF16, tag="w2", name="w2")
            nc.scalar.dma_start(w2t[:], t["w2_d"][k2 * P:(k2 + 1) * P, :])
            w2ts[k2] = w2t

        gTp_cm = tc.tile_pool(name="gTp", bufs=1)
        gtp = gTp_cm.__enter__()
        gT = [gtp.tile([P, TS], BF16, tag=f"gT{mf}", name=f"gT{mf}")
              for mf in range(KF)]
        h2Tp_cm = tc.tile_pool(name="h2Tp", bufs=1)
        h2tp = h2Tp_cm.__enter__()
        h2T = [h2tp.tile([P, TS], BF16, tag=f"h2T{k}", name=f"h2T{k}")
               for k in range(KD)]

        with tc.tile_pool(name="rsb", bufs=1) as rsb, \
             tc.tile_pool(name="lnD", bufs=2) as sc2, \
             tc.tile_pool(name="h2P", bufs=1) as h2sp, \
             tc.tile_pool(name="tpD", bufs=2, space="PSUM") as tpD:
            rstt = rsb.tile([P, NT, D], BF16, tag="rst", name="rst")
            rst = [rstt[:, mt, :] for mt in range(NT)]
            nc.sync.dma_start(rstt[:],
                              t["rs_out"].rearrange("(i p) d -> p i d", p=P))
            for mt in range(NT):
                ve = nc.vector if mt % 2 == 0 else nc.gpsimd
                ve.tensor_tensor(out=xo[mt][:], in0=xo[mt][:], in1=rst[mt][:],
                                 op=ALU.add)
                ve.tensor_tensor(out=xo[mt][:], in0=xo[mt][:], in1=bob[:],
                                 op=ALU.add)
            sqD = h2sp.tile([P, D], F32, tag="sqD", name="sqD")
            h2 = [h2sp.tile([P, D], BF16, tag=f"h2{mt}", name=f"h2{mt}")
                  for mt in range(NT)]
            layernorm(xo, h2, sc2, sqD)
            for k in range(KD):
                transpose_strip(h2, h2T[k][:], ln2g, ln2b, tpD, k, 0, NT,
                                evict_act=(k % 2 == 1))

        # ---- fc1 + gelu ----
        with tc.tile_pool(name="gPS", bufs=4, space="PSUM") as gps:
            for mf in range(KF):
                ps = gps.tile([P, TS], F32, tag="g", name="g")
                for k in range(KD):
                    nc.tensor.matmul(
                        ps[:], w1s[k][:, mf * P:(mf + 1) * P], h2T[k][:],
                        start=(k == 0), stop=(k == KD - 1))
                nc.scalar.activation(gT[mf][:], ps[:], AF.Gelu,
                                     bias=b1p[:, mf:mf + 1])
        h2Tp_cm.__exit__(None, None, None)

        # ---- fc2 + residual ----
        with tc.tile_pool(name="fPS", bufs=1, space="PSUM") as fps, \
             tc.tile_pool(name="ySB", bufs=2) as ysb:
            f_ps = [fps.tile([P, 2, TS], F32, tag=f"f{mt}", name=f"f{mt}")
                    for mt in range(NT)]
            for mt in range(NT):
                for n in range(2):
                    nc.tensor.matmul(
                        f_ps[mt][:, n, :], ones128[:],
                        b2r[:, n * TS:(n + 1) * TS], start=True, stop=False)
            for k2 in range(KF):
                if k2 in w2ts:
                    w2t = w2ts[k2]
                else:
                    w2t = w2p.tile([P, D], BF16, tag="w2", name="w2")
                    nc.scalar.dma_start(w2t[:],
                                        t["w2_d"][k2 * P:(k2 + 1) * P, :])
                for mt in range(NT):
                    for n in range(2):
                        nc.tensor.matmul(
                            f_ps[mt][:, n, :],
                            gT[k2][:, mt * P:(mt + 1) * P],
                            w2t[:, n * TS:(n + 1) * TS],
                            start=False, stop=(k2 == KF - 1))
            for mt in range(NT):
                yt = ysb.tile([P, D], F32, tag="y", name="y")
                nc.vector.tensor_tensor(
                    out=yt[:], in0=f_ps[mt][:].rearrange("p a b -> p (a b)"),
                    in1=xo[mt][:], op=ALU.add)
                nc.sync.dma_start(t["y_d"][mt * P:(mt + 1) * P, :], yt[:])
        gTp_cm.__exit__(None, None, None)
        w2p_cm.__exit__(None, None, None)
        w1p_cm.__exit__(None, None, None)
        xop_cm.__exit__(None, None, None)


def _in_maps(inputs):
    f32 = np.float32
    bf16 = ml_dtypes.bfloat16

    def as_bf16(a):
        return np.ascontiguousarray(np.asarray(a, f32).astype(bf16))

    x = np.asarray(inputs["x"], f32)
    # causal masks for the 4 diagonal sub-blocks: keep where col >= p + 128*d
    cmask = np.zeros((P, 4, 2 * TS), np.float32)
    cols = np.arange(TS)[None, :]
    rows = np.arange(P)[:, None]
    for d in range(4):
        m = (cols >= rows + 128 * d).astype(np.float32)
        cmask[:, d, 0:TS] = m
        cmask[:, d, TS:2 * TS] = m
    cmask = cmask.astype(bf16)
    maps = []
    for c in range(NC):
        b, r = c // G, c % G
        c0 = r * CC
        m = {
            "x": np.ascontiguousarray(x[b]),
            "xo": np.ascontiguousarray(x[b, r * TS:(r + 1) * TS, :]),
            "ln1_g": np.ascontiguousarray(inputs["ln1_g"], f32),
            "ln1_b": np.ascontiguousarray(inputs["ln1_b"], f32),
            "Wq": as_bf16(np.asarray(inputs["Wq"], f32)[:, c0:c0 + CC]),
            "Wk": as_bf16(np.asarray(inputs["Wk"], f32)[:, c0:c0 + CC]),
            "Wv": as_bf16(np.asarray(inputs["Wv"], f32)[:, c0:c0 + CC]),
            "bq": np.ascontiguousarray(np.asarray(inputs["bq"], f32)[c0:c0 + CC]),
            "bk": np.ascontiguousarray(np.asarray(inputs["bk"], f32)[c0:c0 + CC]),
            "bv": np.ascontiguousarray(np.asarray(inputs["bv"], f32)[c0:c0 + CC]),
            "Wo": as_bf16(np.asarray(inputs["Wo"], f32)[c0:c0 + CC, :]),
            "bo": np.ascontiguousarray(inputs["bo"], f32),
            "ln2_g": np.ascontiguousarray(inputs["ln2_g"], f32),
            "ln2_b": np.ascontiguousarray(inputs["ln2_b"], f32),
            "W1": as_bf16(inputs["W1"]),
            "b1": np.ascontiguousarray(inputs["b1"], f32),
            "W2": as_bf16(inputs["W2"]),
            "b2": np.ascontiguousarray(inputs["b2"], f32),
            "cmask": cmask,
        }
        maps.append(m)
    return maps


def _run(inputs, trace=False):
    if "nc" not in _CACHE:
        _CACHE["nc"] = build()
    nc = _CACHE["nc"]
    maps = _in_maps(inputs)
    res = run_bass_kernel_spmd(nc, maps, list(range(NC)), trace=trace)
    out = np.empty((B, S, D), np.float32)
    for c in range(NC):
        b, r = c // G, c % G
        out[b, r * TS:(r + 1) * TS, :] = res.results[c]["y"]
    return out, res


def kernel(**inputs):
    out, _ = _run(inputs, trace=False)
    return out


if __name__ == "__main__":
    build()
    print("build OK")


# revision 15
# speedup vs baseline: 1.8713x; 1.0005x over previous
"""Trainium2 Bass kernel for a dense transformer block (B=2,S=2048,D=1024,H=16,DFF=4096).

Sharding across 8 NeuronCores:
  core c: batch b=c//4, group rank r=c%4, replica groups [[0,1,2,3],[4,5,6,7]].
  - Every core loads the FULL x of its batch; LN1 + transpose are replicated
    (no collective needed for the attention input).
  - Attention: head-parallel (4 heads/core, full causal sequence), output
    kept on-chip.
  - out_proj: each core computes the partial sum over its own heads for ALL
    2048 tokens; a single ReduceScatter(add) in bf16 then hands each core its
    own summed 512-token strip (output bytes 1MB vs the 8MB an AllGather of
    the attention output would move).
  - residual, LN2, FFN: token-sharded (512 tokens/core), full weights.
Matmuls run in bf16 (psum accumulation stays fp32); LN statistics and the
residual spine stay fp32.
"""
import sys

sys.path.insert(0, "/opt/trn_rl_repo")

import numpy as np
import ml_dtypes

import concourse.bass as bass
import concourse.mybir as mybir
import concourse.tile as tile
from concourse import bacc
from concourse.bass_utils import run_bass_kernel_spmd
from concourse.masks import make_identity

AF = mybir.ActivationFunctionType
ALU = mybir.AluOpType
F32 = mybir.dt.float32
F32R = mybir.dt.float32r
BF16 = mybir.dt.bfloat16

B, S, D, H = 2, 2048, 1024, 16
DH = D // H          # 64
DFF = 4 * D          # 4096
EPS = 1e-5
NC = 8               # cores
G = 4                # cores per group (per batch)
TS = S // G          # 512 tokens per strip
HC = H // G          # 4 heads per core
CC = HC * DH         # 256 head-columns per core
P = 128
KD = D // P          # 8 k-tiles over D
KF = DFF // P        # 32 k-tiles over DFF
NTT = S // P         # 16 token tiles (full sequence)
NT = TS // P         # 4 token tiles per strip
GROUPS = [[0, 1, 2, 3], [4, 5, 6, 7]]

_CACHE = {}


def build():
    nc = bacc.Bacc(None)

    io = {}
    io["x_d"] = nc.declare_dram_parameter("x", [S, D], F32, isOutput=False)
    io["xo_d"] = nc.declare_dram_parameter("xo", [TS, D], F32, isOutput=False)
    io["ln1g_d"] = nc.declare_dram_parameter("ln1_g", [D], F32, isOutput=False)
    io["ln1b_d"] = nc.declare_dram_parameter("ln1_b", [D], F32, isOutput=False)
    io["wq_d"] = nc.declare_dram_parameter("Wq", [D, CC], BF16, isOutput=False)
    io["wk_d"] = nc.declare_dram_parameter("Wk", [D, CC], BF16, isOutput=False)
    io["wv_d"] = nc.declare_dram_parameter("Wv", [D, CC], BF16, isOutput=False)
    io["bq_d"] = nc.declare_dram_parameter("bq", [CC], F32, isOutput=False)
    io["bk_d"] = nc.declare_dram_parameter("bk", [CC], F32, isOutput=False)
    io["bv_d"] = nc.declare_dram_parameter("bv", [CC], F32, isOutput=False)
    io["wo_d"] = nc.declare_dram_parameter("Wo", [CC, D], BF16, isOutput=False)
    io["bo_d"] = nc.declare_dram_parameter("bo", [D], F32, isOutput=False)
    io["ln2g_d"] = nc.declare_dram_parameter("ln2_g", [D], F32, isOutput=False)
    io["ln2b_d"] = nc.declare_dram_parameter("ln2_b", [D], F32, isOutput=False)
    io["w1_d"] = nc.declare_dram_parameter("W1", [D, DFF], BF16, isOutput=False)
    io["b1_d"] = nc.declare_dram_parameter("b1", [DFF], F32, isOutput=False)
    io["w2_d"] = nc.declare_dram_parameter("W2", [DFF, D], BF16, isOutput=False)
    io["b2_d"] = nc.declare_dram_parameter("b2", [D], F32R, isOutput=False)
    io["cmask_d"] = nc.declare_dram_parameter("cmask", [P, 4, 2 * TS], BF16,
                                              isOutput=False)
    io["y_d"] = nc.declare_dram_parameter("y", [TS, D], F32, isOutput=True)

    io["rs_in"] = nc.dram_tensor("rs_in", [S, D], BF16)
    io["rs_out"] = nc.dram_tensor("rs_out", [TS, D], BF16)

    with tile.TileContext(nc) as tc:
        _body(nc, tc, io)
    nc.compile()
    return nc


def _body(nc, tc, t):
    with tc.tile_pool(name="const", bufs=1) as cst:
        # ---- tiny constants + the LN1 params gate phase A: issue first ----
        ln1g = cst.tile([P, KD], F32)
        ln1b = cst.tile([P, KD], F32)
        nc.sync.dma_start(ln1g[:], t["ln1g_d"].rearrange("(k p) -> p k", p=P))
        nc.sync.dma_start(ln1b[:], t["ln1b_d"].rearrange("(k p) -> p k", p=P))

        identb = cst.tile([P, P], BF16)
        make_identity(nc, identb[:])
        epsc = cst.tile([P, 1], F32)
        nc.gpsimd.memset(epsc[:], EPS)
        onesrow_f = cst.tile([1, P], F32)
        nc.gpsimd.memset(onesrow_f[:], 1.0)
        ones128 = cst.tile([1, P], F32R)
        nc.vector.tensor_copy(ones128[:], onesrow_f[:])
        onescol4 = cst.tile([P, HC, 1], F32)
        nc.gpsimd.memset(onescol4[:], 1.0)

        # ---------------- helpers ----------------
        def layernorm(src_tiles, dst_tiles, sc, sq):
            # var = E[x^2] - mu^2; normalize is one fused (x - mu) * inv pass.
            # reduce/normalize alternate between DVE and Pool to halve the
            # DVE serial time; the Square/Sqrt chain stays on Act.
            n = len(src_tiles)
            for mt in range(n):
                ve = nc.vector if mt % 2 == 0 else nc.gpsimd
                xt = src_tiles[mt]
                mu = sc.tile([P, 1], F32, tag="mu", name="mu")
                nc.vector.tensor_reduce(out=mu[:], in_=xt[:], op=ALU.add,
                                        axis=mybir.AxisListType.X)
                mus = sc.tile([P, 1], F32, tag="mus", name="mus")
                nc.scalar.mul(mus[:], mu[:], 1.0 / D)
                sumsq = sc.tile([P, 1], F32, tag="sumsq", name="sumsq")
                nc.scalar.activation(sq[:], xt[:], AF.Square, accum_out=sumsq[:])
                mu2 = sc.tile([P, 1], F32, tag="mu2", name="mu2")
                nc.scalar.activation(mu2[:], mus[:], AF.Square)
                vpe = sc.tile([P, 1], F32, tag="vpe", name="vpe")
                ve.tensor_scalar(out=vpe[:], in0=sumsq[:],
                                 scalar1=1.0 / D, scalar2=mu2[:],
                                 op0=ALU.mult, op1=ALU.subtract)
                std = sc.tile([P, 1], F32, tag="std", name="std")
                nc.scalar.activation(std[:], vpe[:], AF.Sqrt, bias=epsc[:])
                inv = sc.tile([P, 1], F32, tag="inv", name="inv")
                nc.vector.reciprocal(inv[:], std[:])
                ve.tensor_scalar(out=dst_tiles[mt][:], in0=xt[:],
                                 scalar1=mus[:], scalar2=inv[:],
                                 op0=ALU.subtract, op1=ALU.mult)

        def transpose_strip(h_tiles, dst, g_sb, b_sb, tp, k, base_mt, nmt,
                            evict_act):
            # transpose nmt 128x128 blocks of column-tile k into one psum
            # bank, then evict once with the fused *g+b (DVE or Act).
            ps = tp.tile([P, NT * P], BF16, tag="tps", name="tps")
            for j in range(nmt):
                nc.tensor.transpose(ps[:, j * P:(j + 1) * P],
                                    h_tiles[base_mt + j][:, k * P:(k + 1) * P],
                                    identb[:])
            w = nmt * P
            if evict_act:
                nc.scalar.activation(dst[:, :w], ps[:, :w], AF.Identity,
                                     bias=b_sb[:, k:k + 1],
                                     scale=g_sb[:, k:k + 1])
            else:
                nc.vector.tensor_scalar(out=dst[:, :w], in0=ps[:, :w],
                                        scalar1=g_sb[:, k:k + 1],
                                        scalar2=b_sb[:, k:k + 1],
                                        op0=ALU.mult, op1=ALU.add)

        # wo + masks live in the const pool
        wot = cst.tile([P, 2, D], BF16, tag="wot", name="wot")
        wo = [wot[:, kk, :] for kk in range(2)]
        cmaskt = cst.tile([P, 4, 2 * TS], BF16, tag="cmask", name="cmask")

        # ============ persistent pools (LIFO lifetimes) ============
        xop_cm = tc.tile_pool(name="xop", bufs=1)
        xop = xop_cm.__enter__()
        xot = xop.tile([P, NT, D], F32, tag="xot", name="xot")
        xo = [xot[:, mt, :] for mt in range(NT)]

        w1p_cm = tc.tile_pool(name="w1p", bufs=1)
        w1p = w1p_cm.__enter__()
        w1t = w1p.tile([P, KD, DFF], BF16, tag="w1t", name="w1t")
        w1s = [w1t[:, k, :] for k in range(KD)]

        qkvp_cm = tc.tile_pool(name="qkvP", bufs=1)
        qkv = qkvp_cm.__enter__()
        qT = [qkv.tile([P, S], BF16, tag=f"qT{m}", name=f"qT{m}") for m in range(2)]
        kT = [qkv.tile([P, S], BF16, tag=f"kT{m}", name=f"kT{m}") for m in range(2)]
        vo = [qkv.tile([P, HC, DH + 1], BF16, tag=f"vo{tm}", name=f"vo{tm}")
              for tm in range(NTT)]
        aT = [qkv.tile([P, S], BF16, tag=f"aT{kk}", name=f"aT{kk}")
              for kk in range(2)]

        wqkvp_cm = tc.tile_pool(name="wqkv", bufs=1)
        wqkvp = wqkvp_cm.__enter__()
        wqt = wqkvp.tile([P, KD, CC], BF16, tag="wqt", name="wqt")
        wkt = wqkvp.tile([P, KD, CC], BF16, tag="wkt", name="wkt")
        wvt = wqkvp.tile([P, KD, CC], BF16, tag="wvt", name="wvt")
        wq = [wqt[:, k, :] for k in range(KD)]
        wk = [wkt[:, k, :] for k in range(KD)]
        wv = [wvt[:, k, :] for k in range(KD)]

        hTp_cm = tc.tile_pool(name="hTp", bufs=1)
        hTp = hTp_cm.__enter__()
        hT = [hTp.tile([P, S], BF16, tag=f"hT{k}", name=f"hT{k}")
              for k in range(KD)]

        # ============ phase A: x stream + LN1 + transpose ============
        with tc.tile_pool(name="xs", bufs=2) as xsp, \
             tc.tile_pool(name="hs", bufs=1) as hsp, \
             tc.tile_pool(name="sqA", bufs=1) as sqp, \
             tc.tile_pool(name="lnA", bufs=2) as sc, \
             tc.tile_pool(name="tpA", bufs=2, space="PSUM") as tpA:
            sqA = sqp.tile([P, D], F32, tag="sqA", name="sqA")
            for mtg in range(NTT // NT):
                hcur = []
                for j in range(NT):
                    mt = mtg * NT + j
                    xt = xsp.tile([P, D], F32, tag="x", name="x")
                    nc.sync.dma_start(xt[:], t["x_d"][mt * P:(mt + 1) * P, :])
                    ht = hsp.tile([P, D], BF16, tag=f"h{j}", name=f"h{j}")
                    layernorm([xt], [ht], sc, sqA)
                    hcur.append(ht)
                for k in range(KD):
                    transpose_strip(hcur,
                                    hT[k][:, mtg * NT * P:(mtg + 1) * NT * P],
                                    ln1g, ln1b, tpA, k, 0, NT,
                                    evict_act=(k % 2 == 1))
                if mtg == 0:
                    # QKV weights gate phase B: issue early, they slot into
                    # gaps of the LN-paced x stream (small transfers)
                    nc.sync.dma_start(
                        wqt[:], t["wq_d"].rearrange("(k p) c -> p k c", p=P))
                    nc.sync.dma_start(
                        wkt[:], t["wk_d"].rearrange("(k p) c -> p k c", p=P))
                    nc.sync.dma_start(
                        wvt[:], t["wv_d"].rearrange("(k p) c -> p k c", p=P))

            # needed only from attention / out_proj / FFN onward: the DMA
            # device serves by readiness, so keep chunks small and late
            nc.sync.dma_start(wot[:], t["wo_d"].rearrange("(k p) d -> p k d", p=P))
            nc.sync.dma_start(cmaskt[:], t["cmask_d"][:])
            for i in range(NT):
                nc.sync.dma_start(xot[:, i, :], t["xo_d"][i * P:(i + 1) * P, :])
            for k in range(KD):
                nc.sync.dma_start(w1t[:, k, :], t["w1_d"][k * P:(k + 1) * P, :])

        # remaining small constants (issued behind x on SP; needed later)
        ln2g = cst.tile([P, KD], F32)
        ln2b = cst.tile([P, KD], F32)
        nc.sync.dma_start(ln2g[:], t["ln2g_d"].rearrange("(k p) -> p k", p=P))
        nc.sync.dma_start(ln2b[:], t["ln2b_d"].rearrange("(k p) -> p k", p=P))
        bqp = cst.tile([P, 2], F32)
        bkp = cst.tile([P, 2], F32)
        nc.sync.dma_start(bqp[:], t["bq_d"].rearrange("(m p) -> p m", p=P))
        nc.sync.dma_start(bkp[:], t["bk_d"].rearrange("(m p) -> p m", p=P))
        bvrow = cst.tile([1, CC], F32)
        nc.sync.dma_start(bvrow[:], t["bv_d"][None, :])
        bvb = cst.tile([P, CC], F32)
        nc.gpsimd.partition_broadcast(bvb[:], bvrow[:])
        borow = cst.tile([1, D], F32)
        nc.sync.dma_start(borow[:], t["bo_d"][None, :])
        bob = cst.tile([P, D], F32)
        nc.gpsimd.partition_broadcast(bob[:], borow[:])
        b1p = cst.tile([P, KF], F32)
        nc.sync.dma_start(b1p[:], t["b1_d"].rearrange("(k p) -> p k", p=P))
        b2r = cst.tile([1, D], F32R)
        nc.sync.dma_start(b2r[:], t["b2_d"][None, :])


        # ============ phase B: QKV projections ============
        with tc.tile_pool(name="projPS", bufs=2, space="PSUM") as pps, \
             tc.tile_pool(name="vPS", bufs=2, space="PSUM") as vps:
            for (w_sb, b_sb, out_sb) in ((wq, bqp, qT), (wk, bkp, kT)):
                for m in range(2):
                    for st in range(G):
                        ps = pps.tile([P, TS], F32, tag="pp", name="pp")
                        for k in range(KD):
                            nc.tensor.matmul(
                                ps[:], w_sb[k][:, m * P:(m + 1) * P],
                                hT[k][:, st * TS:(st + 1) * TS],
                                start=(k == 0), stop=(k == KD - 1))
                        if st % 2 == 0:
                            nc.vector.tensor_scalar(
                                out=out_sb[m][:, st * TS:(st + 1) * TS],
                                in0=ps[:], scalar1=b_sb[:, m:m + 1],
                                scalar2=None, op0=ALU.add)
                        else:
                            nc.scalar.activation(
                                out_sb[m][:, st * TS:(st + 1) * TS], ps[:],
                                AF.Identity, bias=b_sb[:, m:m + 1])
            for tm in range(NTT):
                ps = vps.tile([P, CC], F32, tag="vp", name="vp")
                for k in range(KD):
                    nc.tensor.matmul(
                        ps[:], hT[k][:, tm * P:(tm + 1) * P], wv[k][:],
                        start=(k == 0), stop=(k == KD - 1))
                nc.vector.tensor_tensor(
                    out=vo[tm][:, :, 0:DH],
                    in0=ps[:].rearrange("p (h e) -> p h e", h=HC),
                    in1=bvb[:].rearrange("p (h e) -> p h e", h=HC),
                    op=ALU.add)
                nc.vector.tensor_copy(vo[tm][:, :, DH:DH + 1], onescol4[:])

        hTp_cm.__exit__(None, None, None)
        wqkvp_cm.__exit__(None, None, None)

        # ===== phases C+D: attention (qc-outer) + fused out_proj =====
        # per query strip: both head pairs' attention, then that strip's
        # out_proj partial immediately (fills PE while Act runs exp).
        # kt loop is software-pipelined: scores(kt+1) is traced before AV(kt)
        # so PE isn't idle while Act computes exp(kt).
        with (
            tc.tile_pool(name="scPS", bufs=2, space="PSUM") as scp,
            tc.tile_pool(name="avPS", bufs=1, space="PSUM") as avp,
            tc.tile_pool(name="opPS", bufs=1, space="PSUM") as opp,
            tc.tile_pool(name="attnSB", bufs=3) as asb,
            tc.tile_pool(name="opSB", bufs=3) as osb,
        ):
            for qc in range(G):
                kt_max = 4 * qc + 3
                for hp in range(HC // 2):      # head pairs at PE rows 0/64
                    avs = [avp.tile([DH + 1, TS], F32, tag=f"av{j}",
                                    name=f"av{j}") for j in range(2)]

                    def scores(kt):
                        w0 = P * max(0, kt - 4 * qc)
                        sc_ps = scp.tile([P, 2, TS], F32, tag="scp", name="scp")
                        for j in range(2):
                            o = j * DH
                            nc.tensor.matmul(
                                sc_ps[:, j, w0:],
                                kT[hp][o:o + DH, kt * P:(kt + 1) * P],
                                qT[hp][o:o + DH, qc * TS + w0:(qc + 1) * TS],
                                start=True, stop=True)
                        return sc_ps

                    def expmask(kt, sc_ps):
                        e_r = asb.tile([P, 2, TS], BF16, tag="erp", name="erp")
                        if kt < 4 * qc:
                            v0 = 0      # valid columns start
                            nc.scalar.activation(
                                e_r[:].rearrange("p a b -> p (a b)"),
                                sc_ps[:].rearrange("p a b -> p (a b)"),
                                AF.Exp, scale=0.125)
                        else:
                            # diag block d: cols < 128*d are fully masked --
                            # never compute/read them
                            d = kt - 4 * qc
                            v0 = P * d
                            e_f = asb.tile([P, 2, TS], BF16, tag="efp",
                                           name="efp")
                            nc.scalar.activation(
                                e_f[:, :, v0:], sc_ps[:, :, v0:],
                                AF.Exp, scale=0.125)
                            mdv = cmaskt[:, d, :].rearrange(
                                "p (a b) -> p a b", a=2)
                            nc.vector.tensor_tensor(
                                out=e_r[:, :, v0:], in0=e_f[:, :, v0:],
                                in1=mdv[:, :, v0:], op=ALU.mult)
                        return e_r, v0

                    sc_prev = scores(0)
                    for kt in range(kt_max + 1):
                        e_r, v0 = expmask(kt, sc_prev)
                        if kt < kt_max:
                            sc_prev = scores(kt + 1)
                        for j in range(2):
                            nc.tensor.matmul(avs[j][:, v0:],
                                             vo[kt][:, 2 * hp + j, :],
                                             e_r[:, j, v0:],
                                             start=(kt == 0),
                                             stop=(kt == kt_max))
                    for j in range(2):
                        rec = asb.tile([1, TS], F32, tag=f"rec{j}",
                                       name=f"rec{j}")
                        nc.vector.reciprocal(rec[:], avs[j][DH:DH + 1, :])
                        rb = asb.tile([DH, TS], F32, tag=f"rb{j}",
                                      name=f"rb{j}")
                        nc.gpsimd.partition_broadcast(rb[:], rec[:])
                        nc.vector.tensor_tensor(
                            out=aT[hp][j * DH:(j + 1) * DH,
                                       qc * TS:(qc + 1) * TS],
                            in0=avs[j][0:DH, :], in1=rb[:], op=ALU.mult)

                # out_proj partial for this strip's tokens
                for mt in range(qc * NT, (qc + 1) * NT):
                    ps = opp.tile([P, 2, TS], F32, tag="op", name="op")
                    for n in range(2):
                        for kk in range(2):
                            nc.tensor.matmul(
                                ps[:, n, :], aT[kk][:, mt * P:(mt + 1) * P],
                                wo[kk][:, n * TS:(n + 1) * TS],
                                start=(kk == 0), stop=(kk == 1))
                    ot = osb.tile([P, D], BF16, tag="ot", name="ot")
                    if mt % 2 == 0:
                        nc.vector.tensor_copy(
                            ot[:], ps[:].rearrange("p a b -> p (a b)"))
                    else:
                        nc.scalar.activation(
                            ot[:], ps[:].rearrange("p a b -> p (a b)"), AF.Copy)
                    nc.sync.dma_start(t["rs_in"][mt * P:(mt + 1) * P, :], ot[:])

        qkvp_cm.__exit__(None, None, None)

        # ============ ReduceScatter: sum head-partials, keep own strip ======
        nc.gpsimd.collective_compute(
            "ReduceScatter", ALU.add, ins=[t["rs_in"][:]],
            outs=[t["rs_out"][:]], replica_groups=GROUPS,
        )

        # ============ phase E: residual + LN2 + FFN (token-sharded) ========
        # W2 stream pool opens first (LIFO: closes last); prefetch half of W2
        # on the Act DGE queue so it transfers during the ReduceScatter.
        w2p_cm = tc.tile_pool(name="w2st", bufs=16)
        w2p = w2p_cm.__enter__()
        w2ts = {}
        for k2 in range(KF // 2):
            w2t = w2p.tile([P, D], BF16, tag="w2", name="w2")
            nc.scalar.dma_start(w2t[:], t["w2_d"][k2 * P:(k2 + 1) * P, :])
            w2ts[k2] = w2t

        gTp_cm = tc.tile_pool(name="gTp", bufs=1)
        gtp = gTp_cm.__enter__()
        gT = [gtp.tile([P, TS], BF16, tag=f"gT{mf}", name=f"gT{mf}")
              for mf in range(KF)]
        h2Tp_cm = tc.tile_pool(name="h2Tp", bufs=1)
        h2tp = h2Tp_cm.__enter__()
        h2T = [h2tp.tile([P, TS], BF16, tag=f"h2T{k}", name=f"h2T{k}")
               for k in range(KD)]

        with tc.tile_pool(name="rsb", bufs=1) as rsb, \
             tc.tile_pool(name="lnD", bufs=2) as sc2, \
             tc.tile_pool(name="h2P", bufs=1) as h2sp, \
             tc.tile_pool(name="tpD", bufs=2, space="PSUM") as tpD:
            rstt = rsb.tile([P, NT, D], BF16, tag="rst", name="rst")
            rst = [rstt[:, mt, :] for mt in range(NT)]
            nc.sync.dma_start(rstt[:],
                              t["rs_out"].rearrange("(i p) d -> p i d", p=P))
            for mt in range(NT):
                ve = nc.vector if mt % 2 == 0 else nc.gpsimd
                ve.tensor_tensor(out=xo[mt][:], in0=xo[mt][:], in1=rst[mt][:],
                                 op=ALU.add)
                ve.tensor_tensor(out=xo[mt][:], in0=xo[mt][:], in1=bob[:],
                                 op=ALU.add)
            sqD = h2sp.tile([P, D], F32, tag="sqD", name="sqD")
            h2 = [h2sp.tile([P, D], BF16, tag=f"h2{mt}", name=f"h2{mt}")
                  for mt in range(NT)]
            layernorm(xo, h2, sc2, sqD)
            for k in range(KD):
                transpose_strip(h2, h2T[k][:], ln2g, ln2b, tpD, k, 0, NT,
                                evict_act=(k % 2 == 1))

        # ---- fc1 + gelu ----
        with tc.tile_pool(name="gPS", bufs=4, space="PSUM") as gps:
            for mf in range(KF):
                ps = gps.tile([P, TS], F32, tag="g", name="g")
                for k in range(KD):
                    nc.tensor.matmul(
                        ps[:], w1s[k][:, mf * P:(mf + 1) * P], h2T[k][:],
                        start=(k == 0), stop=(k == KD - 1))
                nc.scalar.activation(gT[mf][:], ps[:], AF.Gelu,
                                     bias=b1p[:, mf:mf + 1])
        h2Tp_cm.__exit__(None, None, None)

        # ---- fc2 + residual ----
        with tc.tile_pool(name="fPS", bufs=1, space="PSUM") as fps, \
             tc.tile_pool(name="ySB", bufs=2) as ysb:
            f_ps = [fps.tile([P, 2, TS], F32, tag=f"f{mt}", name=f"f{mt}")
                    for mt in range(NT)]
            for mt in range(NT):
                for n in range(2):
                    nc.tensor.matmul(
                        f_ps[mt][:, n, :], ones128[:],
                        b2r[:, n * TS:(n + 1) * TS], start=True, stop=False)
            for k2 in range(KF):
                if k2 in w2ts:
                    w2t = w2ts[k2]
                else:
                    w2t = w2p.tile([P, D], BF16, tag="w2", name="w2")
                    nc.scalar.dma_start(w2t[:],
                                        t["w2_d"][k2 * P:(k2 + 1) * P, :])
                for mt in range(NT):
                    for n in range(2):
                        nc.tensor.matmul(
                            f_ps[mt][:, n, :],
                            gT[k2][:, mt * P:(mt + 1) * P],
                            w2t[:, n * TS:(n + 1) * TS],
                            start=False, stop=(k2 == KF - 1))
            for mt in range(NT):
                yt = ysb.tile([P, D], F32, tag="y", name="y")
                nc.vector.tensor_tensor(
                    out=yt[:], in0=f_ps[mt][:].rearrange("p a b -> p (a b)"),
                    in1=xo[mt][:], op=ALU.add)
                nc.sync.dma_start(t["y_d"][mt * P:(mt + 1) * P, :], yt[:])
        gTp_cm.__exit__(None, None, None)
        w2p_cm.__exit__(None, None, None)
        w1p_cm.__exit__(None, None, None)
        xop_cm.__exit__(None, None, None)


def _in_maps(inputs):
    f32 = np.float32
    bf16 = ml_dtypes.bfloat16

    def as_bf16(a):
        return np.ascontiguousarray(np.asarray(a, f32).astype(bf16))

    x = np.asarray(inputs["x"], f32)
    # causal masks for the 4 diagonal sub-blocks: keep where col >= p + 128*d
    cmask = np.zeros((P, 4, 2 * TS), np.float32)
    cols = np.arange(TS)[None, :]
    rows = np.arange(P)[:, None]
    for d in range(4):
        m = (cols >= rows + 128 * d).astype(np.float32)
        cmask[:, d, 0:TS] = m
        cmask[:, d, TS:2 * TS] = m
    cmask = cmask.astype(bf16)
    maps = []
    for c in range(NC):
        b, r = c // G, c % G
        c0 = r * CC
        m = {
            "x": np.ascontiguousarray(x[b]),
            "xo": np.ascontiguousarray(x[b, r * TS:(r + 1) * TS, :]),
            "ln1_g": np.ascontiguousarray(inputs["ln1_g"], f32),
            "ln1_b": np.ascontiguousarray(inputs["ln1_b"], f32),
            "Wq": as_bf16(np.asarray(inputs["Wq"], f32)[:, c0:c0 + CC]),
            "Wk": as_bf16(np.asarray(inputs["Wk"], f32)[:, c0:c0 + CC]),
            "Wv": as_bf16(np.asarray(inputs["Wv"], f32)[:, c0:c0 + CC]),
            "bq": np.ascontiguousarray(np.asarray(inputs["bq"], f32)[c0:c0 + CC]),
            "bk": np.ascontiguousarray(np.asarray(inputs["bk"], f32)[c0:c0 + CC]),
            "bv": np.ascontiguousarray(np.asarray(inputs["bv"], f32)[c0:c0 + CC]),
            "Wo": as_bf16(np.asarray(inputs["Wo"], f32)[c0:c0 + CC, :]),
            "bo": np.ascontiguousarray(inputs["bo"], f32),
            "ln2_g": np.ascontiguousarray(inputs["ln2_g"], f32),
            "ln2_b": np.ascontiguousarray(inputs["ln2_b"], f32),
            "W1": as_bf16(inputs["W1"]),
            "b1": np.ascontiguousarray(inputs["b1"], f32),
            "W2": as_bf16(inputs["W2"]),
            "b2": np.ascontiguousarray(inputs["b2"], f32),
            "cmask": cmask,
        }
        maps.append(m)
    return maps


def _run(inputs, trace=False):
    if "nc" not in _CACHE:
        _CACHE["nc"] = build()
    nc = _CACHE["nc"]
    maps = _in_maps(inputs)
    res = run_bass_kernel_spmd(nc, maps, list(range(NC)), trace=trace)
    out = np.empty((B, S, D), np.float32)
    for c in range(NC):
        b, r = c // G, c % G
        out[b, r * TS:(r + 1) * TS, :] = res.results[c]["y"]
    return out, res


def kernel(**inputs):
    out, _ = _run(inputs, trace=False)
    return out


if __name__ == "__main__":
    build()
    print("build OK")
